# revision 18
# baseline (speedup 1.0000x reference)
"""Trainium2 Bass kernel for nn_Attention_12137577578573.

Full multi-head attention (QKV projection + masked softmax + context) for
B=4, F=T=2048, CF=CT=1024, H=16, DH=64, sharded over 8 NeuronCores as
(batch b, head-group hg): core i = (b = i // 2, hg = i % 2), each core
computing 1 batch x 8 heads.

v3 schedule (pair-major units for uniform PE load):
  - units iterate u -> (pair = u//4, fc = u%4) instead of (fc, pair).
    With fc-major order ALL K and V projection chains were front-loaded
    into units 0-3 (~23us/unit PE while ACT idled ~30us); pair-major makes
    the projection-chain deadlines uniform: k(pair) chains spread over the
    previous pair-block, v half-chains over units 0-8, one q chain per
    unit.  Per-unit PE load ~15-17us everywhere.
  - cost: all 4 xT quarters stay resident and each unit needs its own fc's
    mask half-tiles.  Masks stay bf16 (an fp8 mask operand drops the DVE
    multiply to 1 elem/cycle: 624 -> 1372 ns/tile, v3a measured) but are
    re-streamed per unit through a 4-buffer ring (2 MB/unit, prefetched one
    unit ahead, split across both HWDGE rings); pT ring shrinks 32->24,
    kT/qT become rings (3 and 8 bufs).
  - softmax exp: ACT for most tiles; OFF_SLOTS tiles use the DVE
    Schraudolph exponent bit-trick written straight into the pT tile
    (bitcast int16); GPS_SLOTS tiles run their mask-multiply on the
    otherwise idle GPSIMD engine instead of DVE.

Layout strategy (unchanged from v1/v2):
  - host pre-transposes from/to tensors -> xT/yT [C, F or T] so QKV
    projections contract C on partitions; Q^T/K^T in transposed layout so
    scores contract DH on partitions; 2 heads packed per 128-partition tile
    (concurrent row-group matmuls).  Scores come out S^T [T, F].  Mask folded
    multiplicatively after exp.  Context C = P^T.T @ [V | 1] gives the
    softmax denominator for free; normalized via per-partition reciprocal.
  - reference reshapes K as (T, DH, H); handled by host-side column
    permutation of Wk/bk.
"""

import sys

if "/opt/trn_rl_repo" not in sys.path:
    sys.path.insert(0, "/opt/trn_rl_repo")

import contextlib

import numpy as np
import ml_dtypes

import concourse.bass as bass
import concourse.bacc as bacc
import concourse.mybir as mybir
import concourse.tile as tile
from concourse import bass_utils

BF16 = mybir.dt.bfloat16
F32 = mybir.dt.float32
F8 = mybir.dt.float8e4
I16 = mybir.dt.int16
bf16 = ml_dtypes.bfloat16
f8 = ml_dtypes.float8_e4m3fn

B, F, T, C, H, DH = 4, 2048, 2048, 1024, 16, 64
HL = 8          # heads per core
COLS = HL * DH  # 512 projected columns per core
ALPHA = 0.125   # 1/sqrt(64)
NCORES = 8
KT = C // 128   # 8 contraction tiles for projections
NFT = F // 128  # 16 F tiles
NTT = T // 128  # 16 T tiles
NU = 16         # units: (pair, fc)

LOG2E = 1.4426950408889634
SCHR_A = ALPHA * LOG2E * 128.0       # score -> bf16-exponent-lattice scale
SCHR_B = 127.0 * 128.0 - 5.0         # exponent bias, mean-error centered

# Schraudolph(DVE)-offloaded exp tile slots per unit (mid-unit so the psum
# drain doesn't stall the next unit's scores).
OFF_SLOTS = [(5, 11)] * NU
# slots whose mask-multiply runs on GPSIMD instead of DVE
GPS_SLOTS = [(1, 7, 13)] * NU

PROFILE = False
LAST_RESULTS = None

_nc_cache = None


def _emit(tc, nc, aps):
    xT, yT, maskT, wq, wk, wv, bq, bk, bv, out = aps
    Exp = mybir.ActivationFunctionType.Exp
    Mult = mybir.AluOpType.mult
    Add = mybir.AluOpType.add

    with contextlib.ExitStack() as ctx:
        pool = ctx.enter_context(tc.tile_pool(name="static", bufs=1))
        qTp = ctx.enter_context(tc.tile_pool(name="qTp", bufs=8))
        kTp = ctx.enter_context(tc.tile_pool(name="kTp", bufs=3))
        maskp = ctx.enter_context(tc.tile_pool(name="maskp", bufs=4))
        pTp = ctx.enter_context(tc.tile_pool(name="pTp", bufs=23))
        outp = ctx.enter_context(tc.tile_pool(name="outp", bufs=1))
        psum_s = ctx.enter_context(tc.tile_pool(name="psum_s", bufs=3, space="PSUM"))
        psum_ctx = ctx.enter_context(tc.tile_pool(name="psum_ctx", bufs=2, space="PSUM"))

        # ---- static tiles ----
        yT_sb = pool.tile([128, KT, T], BF16, name="yT_sb", tag="yT_sb")
        wq_sb = pool.tile([128, KT, COLS], BF16, name="wq_sb", tag="wq_sb")
        wk_sb = pool.tile([128, KT, COLS], BF16, name="wk_sb", tag="wk_sb")
        wv_sb = pool.tile([128, KT, COLS], BF16, name="wv_sb", tag="wv_sb")
        xT_sb = [
            pool.tile([128, KT, 512], BF16, name=f"xT{fc}", tag=f"xT{fc}")
            for fc in range(4)
        ]
        v = [pool.tile([128, HL * 65], BF16, name=f"v{tt}", tag=f"v{tt}") for tt in range(NTT)]
        bq_sb = pool.tile([128, 4], F32, name="bq_sb", tag="bq_sb")
        bk_sb = pool.tile([128, 4], F32, name="bk_sb", tag="bk_sb")
        bv_sb = pool.tile([1, COLS], BF16, name="bv_sb", tag="bv_sb")
        ones_sb = pool.tile([1, 128], BF16, name="ones_sb", tag="ones_sb")

        xT_r = xT.rearrange("(k p) f -> p k f", p=128)
        yT_r = yT.rearrange("(k p) t -> p k t", p=128)
        wq_r = wq.rearrange("(k p) c -> p k c", p=128)
        wk_r = wk.rearrange("(k p) c -> p k c", p=128)
        wv_r = wv.rearrange("(k p) c -> p k c", p=128)
        maskT_r = maskT.rearrange("(tt p) f -> p tt f", p=128)
        out_r = out.rearrange("(g p) c -> p g c", p=128)

        # ---- upfront work: two HWDGE DMA rings + engine warmups ----
        # gpsimd (SWDGE, idle): tiny bias loads
        nc.gpsimd.dma_start(bk_sb[:], bk[:])
        nc.gpsimd.dma_start(bq_sb[:], bq[:])
        nc.gpsimd.dma_start(bv_sb[:], bv[:])
        nc.vector.memset(ones_sb[:], 1.0)
        warm_sb = pool.tile([1, 8], F32, name="warm_sb", tag="warm_sb")
        nc.vector.memset(warm_sb[:], 0.0)
        nc.scalar.activation(warm_sb[:], warm_sb[:], Exp)
        # ones columns of all V tiles, written once
        for tt in range(NTT):
            vview = v[tt].rearrange("p (h c) -> p h c", c=65)
            nc.vector.memset(vview[:, :, 64:65], 1.0)

        # sync ring: K/V path in first-use order
        nc.sync.dma_start(yT_sb[:, :, 0:256], yT_r[:, :, 0:256])
        nc.sync.dma_start(wk_sb[:, :, 0:128], wk_r[:, :, 0:128])
        nc.sync.dma_start(wv_sb[:, :, 0:256], wv_r[:, :, 0:256])
        nc.sync.dma_start(yT_sb[:, :, 256:512], yT_r[:, :, 256:512])
        nc.sync.dma_start(yT_sb[:, :, 512:1024], yT_r[:, :, 512:1024])
        nc.sync.dma_start(yT_sb[:, :, 1024:1536], yT_r[:, :, 1024:1536])
        nc.sync.dma_start(yT_sb[:, :, 1536:2048], yT_r[:, :, 1536:2048])

        # per-unit mask ring: unit u needs fc = u%4's two half-tiles,
        # prefetched one unit ahead, split across the two HWDGE rings
        mask_u = {}

        def load_mask(u, hf, eng):
            fc = u % 4
            mh = maskp.tile([128, 8, 512], BF16, name="mh", tag="mask")
            eng.dma_start(
                mh[:],
                maskT_r[:, hf * 8:(hf + 1) * 8, fc * 512:(fc + 1) * 512],
            )
            mask_u[(u, hf)] = mh

        def dma_mask(u, hf, eng):
            return lambda: load_mask(u, hf, eng)

        # scalar ring (2nd HWDGE): Q path + unit 0's masks.  Everything the
        # scalar ring carries in the prologue delays the ACT engine's first
        # exp (the dma_starts sit in the ACT queue until the ring drains), so
        # keep it to the bare minimum; the rest goes via mid-stream triggers.
        nc.scalar.dma_start(xT_sb[0][:], xT_r[:, :, 0:512])
        nc.scalar.dma_start(wq_sb[:, :, 0:128], wq_r[:, :, 0:128])
        load_mask(0, 0, nc.scalar)
        load_mask(0, 1, nc.scalar)

        def dma_s(dst, src):
            return lambda: nc.scalar.dma_start(dst, src)

        def dma_y(dst, src):
            return lambda: nc.sync.dma_start(dst, src)

        # ---- projection chains as 2-step closures (4 MM quads + evac) ----
        kT_tiles = {}
        qT_tiles = {}

        def k_chain_steps(pair, tcc):
            st = {}
            def half(i):
                if i == 0:
                    if pair not in kT_tiles:
                        kT_tiles[pair] = kTp.tile(
                            [128, T], BF16, name="kT", tag="kT"
                        )
                    st["ps"] = psum_s.tile([128, 1024], F32, name="ps_f", tag="s")[:, 0:512]
                ps = st["ps"]
                for k in range(4 * i, 4 * i + 4):
                    nc.tensor.matmul(
                        ps[:],
                        wk_sb[:, k, pair * 128:(pair + 1) * 128],
                        yT_sb[:, k, tcc * 512:(tcc + 1) * 512],
                        start=(k == 0),
                        stop=(k == KT - 1),
                    )
                if i == 1:
                    nc.vector.tensor_scalar_add(
                        kT_tiles[pair][:, tcc * 512:(tcc + 1) * 512],
                        ps,
                        bk_sb[:, pair:pair + 1],
                    )
            return [lambda i=i: half(i) for i in range(2)]

        def q_chain_steps(fc, cb):
            st = {}
            def half(i):
                if i == 0:
                    qT_tiles[(fc, cb)] = qTp.tile(
                        [128, 512], BF16, name="qTt", tag="qT"
                    )
                    st["ps"] = psum_s.tile([128, 1024], F32, name="ps_f", tag="s")[:, 0:512]
                ps = st["ps"]
                xt = xT_sb[fc]
                for k in range(4 * i, 4 * i + 4):
                    nc.tensor.matmul(
                        ps[:],
                        wq_sb[:, k, cb * 128:(cb + 1) * 128],
                        xt[:, k, :],
                        start=(k == 0),
                        stop=(k == KT - 1),
                    )
                if i == 1:
                    nc.vector.tensor_scalar_add(
                        qT_tiles[(fc, cb)][:], ps, bq_sb[:, cb:cb + 1]
                    )
            return [lambda i=i: half(i) for i in range(2)]

        def v_chain_steps(tt, vh):
            # half-width V chain: head columns vh*256:(vh+1)*256 = pairs
            # 2vh, 2vh+1
            st = {}
            def half(i):
                if i == 0:
                    st["ps"] = psum_s.tile([128, 1024], F32, name="ps_f", tag="s")[:, 0:256]
                ps = st["ps"]
                for k in range(4 * i, 4 * i + 4):
                    nc.tensor.matmul(
                        ps[:],
                        yT_sb[:, k, tt * 128:(tt + 1) * 128],
                        wv_sb[:, k, vh * 256:(vh + 1) * 256],
                        start=(k == 0),
                        stop=False,
                    )
                if i == 1:
                    nc.tensor.matmul(
                        ps[:], ones_sb[0:1, :], bv_sb[0:1, vh * 256:(vh + 1) * 256],
                        start=False, stop=True,
                    )
                    vview = v[tt].rearrange("p (h c) -> p h c", c=65)
                    nc.vector.tensor_copy(
                        vview[:, vh * 4:(vh + 1) * 4, 0:64],
                        ps.rearrange("p (h c) -> p h c", c=64)[:],
                    )
            return [lambda i=i: half(i) for i in range(2)]

        # ---- scores + exp + mask ----
        score_ps = {}
        pT_t = {}

        def emit_score(u, tt):
            pair, fc = u // 4, u % 4
            ps = psum_s.tile([128, 1024], F32, name="ps_s", tag="s")
            qt = qT_tiles[(fc, pair)]
            kt = kT_tiles[pair]
            for hh in range(2):
                nc.tensor.matmul(
                    ps[:, hh * 512:(hh + 1) * 512],
                    kt[hh * 64:(hh + 1) * 64, tt * 128:(tt + 1) * 128],
                    qt[hh * 64:(hh + 1) * 64, :],
                    start=True,
                    stop=True,
                )
            score_ps[(u, tt)] = ps

        def emit_exp_mask(u, tt):
            fc = u % 4
            ps = score_ps.pop((u, tt))
            pt = pTp.tile([128, 1024], BF16, name="pT", tag="pT")
            pT_t[(u, tt)] = pt
            mh = mask_u[(u, tt // 8)]
            m = mh[:, tt % 8:tt % 8 + 1, :].broadcast_to([128, 2, 512])
            o = pt.rearrange("p (h c) -> p h c", c=512)
            if tt in OFF_SLOTS[u]:
                nc.vector.tensor_scalar(
                    pt[:].bitcast(I16), ps[:], SCHR_A, SCHR_B, op0=Mult, op1=Add
                )
            else:
                nc.scalar.activation(pt[:], ps[:], Exp, scale=ALPHA)
            eng = nc.gpsimd if tt in GPS_SLOTS[u] else nc.vector
            eng.tensor_mul(o[:], o[:], m)

        # ---- context: column-major accumulation ----
        ctx_ps = {}

        def emit_ctx_col(cu, c, first, last):
            pair = cu // 4
            if c == first:
                ctx_ps[cu] = {
                    hh: psum_ctx.tile([128, 512], F32, name="pc", tag="pc")
                    for hh in range(2)
                }
            pt = pT_t[(cu, c)]
            for hh in range(2):
                pc = ctx_ps[cu][hh]
                h = pair * 2 + hh
                for ft in range(4):
                    # start clears the whole PSUM bank's has_written bits, so
                    # it must be set on the bank's FIRST matmul only (ft==0);
                    # ft 1-3 of the first column land on cleared flags and
                    # overwrite, later columns accumulate.
                    nc.tensor.matmul(
                        pc[:, ft * 65:ft * 65 + 65],
                        pt[:, hh * 512 + ft * 128:hh * 512 + (ft + 1) * 128],
                        v[c][:, h * 65:(h + 1) * 65],
                        start=(c == first and ft == 0),
                        stop=(c == last and ft == 3),
                        skip_group_check=True,
                    )

        def emit_ctx_evac(cu):
            # raw numerators + denominators out; normalization on host
            pair, fc = cu // 4, cu % 4
            ot = outp.tile([128, 4, 130], F32, name="outt", tag="out")
            for hh in range(2):
                pc = ctx_ps[cu][hh]
                nc.vector.tensor_copy(
                    ot[:, :, hh * 65:(hh + 1) * 65],
                    pc[:, 0:260].rearrange("p (ft c) -> p ft c", c=65)[:],
                )
            ctx_ps.pop(cu)
            for tt in range(NTT):
                pT_t.pop((cu, tt), None)
            # last unit's DMA is latency-critical: use the HWDGE sync ring
            eng = nc.sync if cu == 15 else nc.gpsimd
            eng.dma_start(
                out_r[:, fc * 4:(fc + 1) * 4, pair * 130:(pair + 1) * 130],
                ot[:],
            )

        # ---- schedules ----
        # ctx columns emitted at (unit, slot): list of (cu, col, first, last)
        ctx_sched = {}

        def add_ctx(w, s, cu, c, first=0, last=15):
            ctx_sched.setdefault((w, s), []).append((cu, c, first, last))

        for w in range(1, 15):          # 1-unit lag steady state
            for s in range(NTT):
                add_ctx(w, s, w - 1, s)
        # tail compression: ctx(14)'s sources are all ready at u15 start, so
        # run it 2 cols/slot in u15's first half; evac it at slot 8, freeing
        # the psum_ctx buffers for ctx(15) to run 2 cols/slot in the second
        # half (only cols 14-15 + evac remain after the loop)
        for s in range(8):
            add_ctx(15, s, 14, 2 * s)
            add_ctx(15, s, 14, 2 * s + 1)
        for c in range(14):
            add_ctx(15, 9 + c // 2, 15, c)

        # evac at (unit, slot) — must come after the cu's last (stop) column
        evac_sched = {}
        for cu in range(14):
            evac_sched.setdefault((cu + 1, 15), []).append(cu)
        evac_sched.setdefault((15, 8), []).append(14)

        # filler chains, slot-addressed.  Hard ordering constraints (PE FIFO:
        # a waiting instruction must never depend on later PE work):
        #   k(pair,tcc) fully emitted before unit 4*pair slot 4*tcc's score
        #   q(fc,cb) fully emitted before unit 4*cb+fc slot 0
        #   v(tt,vh) for vh=0 before ctx(0) col tt at unit 1 slot tt;
        #   vh=1 before ctx(8) col tt at unit 9 slot tt
        fillers = {}

        def place(u, slot_chains):
            # slot_chains: list of (chain_steps, slot0, slot1)
            for steps, s0, s1 in slot_chains:
                fillers.setdefault((u, s0), []).append(steps[0])
                fillers.setdefault((u, s1), []).append(steps[1])

        place(0, [(k_chain_steps(0, 1), 0, 1), (v_chain_steps(8, 0), 2, 3),
                  (k_chain_steps(0, 2), 4, 5), (q_chain_steps(1, 0), 6, 7),
                  (k_chain_steps(0, 3), 8, 9), (v_chain_steps(9, 0), 10, 11),
                  (v_chain_steps(10, 0), 12, 13), (v_chain_steps(11, 0), 14, 15)])
        place(1, [(v_chain_steps(12, 0), 0, 1), (v_chain_steps(13, 0), 2, 3),
                  (v_chain_steps(14, 0), 4, 5), (v_chain_steps(15, 0), 6, 7),
                  (q_chain_steps(2, 0), 8, 9)])
        place(2, [(k_chain_steps(1, 0), 0, 1), (k_chain_steps(1, 1), 4, 5),
                  (q_chain_steps(3, 0), 6, 7)])
        place(3, [(k_chain_steps(1, 2), 0, 1), (k_chain_steps(1, 3), 4, 5),
                  (q_chain_steps(0, 1), 6, 7)])
        place(4, [(k_chain_steps(2, 0), 0, 1), (v_chain_steps(0, 1), 2, 3),
                  (q_chain_steps(1, 1), 6, 7), (v_chain_steps(1, 1), 10, 11),
                  (v_chain_steps(2, 1), 12, 13)])
        place(5, [(k_chain_steps(2, 1), 0, 1), (v_chain_steps(3, 1), 2, 3),
                  (q_chain_steps(2, 1), 6, 7), (v_chain_steps(4, 1), 10, 11),
                  (v_chain_steps(5, 1), 12, 13)])
        place(6, [(k_chain_steps(2, 2), 0, 1), (v_chain_steps(6, 1), 2, 3),
                  (q_chain_steps(3, 1), 6, 7), (v_chain_steps(7, 1), 10, 11),
                  (v_chain_steps(8, 1), 12, 13)])
        place(7, [(k_chain_steps(2, 3), 0, 1), (v_chain_steps(9, 1), 2, 3),
                  (q_chain_steps(0, 2), 6, 7), (v_chain_steps(10, 1), 10, 11),
                  (v_chain_steps(11, 1), 12, 13)])
        place(8, [(k_chain_steps(3, 0), 0, 1), (v_chain_steps(12, 1), 2, 3),
                  (v_chain_steps(13, 1), 4, 5), (q_chain_steps(1, 2), 6, 7),
                  (v_chain_steps(14, 1), 10, 11)])
        place(9, [(k_chain_steps(3, 1), 0, 1), (v_chain_steps(15, 1), 2, 3),
                  (q_chain_steps(2, 2), 6, 7)])
        place(10, [(k_chain_steps(3, 2), 0, 1), (q_chain_steps(3, 2), 6, 7)])
        place(11, [(k_chain_steps(3, 3), 0, 1), (q_chain_steps(0, 3), 6, 7)])
        place(12, [(q_chain_steps(1, 3), 6, 7)])
        place(13, [(q_chain_steps(2, 3), 6, 7)])
        place(14, [(q_chain_steps(3, 3), 6, 7)])

        # mid-stream DMA triggers at (unit, slot)
        dma_sched = {
            (0, 0): [dma_s(xT_sb[1][:], xT_r[:, :, 512:1024])],
            (0, 4): [dma_mask(1, 0, nc.scalar),
                     dma_y(wk_sb[:, :, 128:256], wk_r[:, :, 128:256])],
            (0, 6): [dma_mask(1, 1, nc.sync)],
            (0, 8): [dma_s(xT_sb[2][:], xT_r[:, :, 1024:1536])],
            (1, 4): [dma_s(xT_sb[3][:], xT_r[:, :, 1536:2048])],
            (1, 8): [dma_y(wv_sb[:, :, 256:512], wv_r[:, :, 256:512])],
            (2, 8): [dma_y(wk_sb[:, :, 256:384], wk_r[:, :, 256:384])],
            (2, 4): [dma_s(wq_sb[:, :, 128:256], wq_r[:, :, 128:256])],
            (5, 8): [dma_y(wk_sb[:, :, 384:512], wk_r[:, :, 384:512])],
            (6, 4): [dma_s(wq_sb[:, :, 256:384], wq_r[:, :, 256:384])],
            (9, 4): [dma_s(wq_sb[:, :, 384:512], wq_r[:, :, 384:512])],
        }
        # prefetch unit u+1's masks during unit u (one tile per ring)
        for u in range(1, 15):
            dma_sched.setdefault((u, 2), []).append(dma_mask(u + 1, 0, nc.scalar))
            dma_sched.setdefault((u, 6), []).append(dma_mask(u + 1, 1, nc.sync))

        # ---- PE warmup: keep the clock warm during the initial DMA wait ----
        warm_ps = psum_s.tile([128, 1024], F32, name="ps_w", tag="s")
        for _ in range(32):
            nc.tensor.matmul(
                warm_ps[:, 0:128], ones_sb[0:1, :], ones_sb[0:1, :],
                start=True, stop=True,
            )

        # ---- prologue compute: k(0) first half in T-eighths (tt 0-3) so the
        # first score tile only needs the first yT/wk chunks, q(0,0), and the
        # first half of vh0 ----
        def k8_chain(tc8):
            ps = psum_s.tile([128, 1024], F32, name="ps_f", tag="s")[:, 0:256]
            if 0 not in kT_tiles:
                kT_tiles[0] = kTp.tile([128, T], BF16, name="kT", tag="kT")
            for k in range(KT):
                nc.tensor.matmul(
                    ps[:],
                    wk_sb[:, k, 0:128],
                    yT_sb[:, k, tc8 * 256:(tc8 + 1) * 256],
                    start=(k == 0),
                    stop=(k == KT - 1),
                )
            nc.vector.tensor_scalar_add(
                kT_tiles[0][:, tc8 * 256:(tc8 + 1) * 256], ps, bk_sb[:, 0:1]
            )

        k8_chain(0)
        for st in q_chain_steps(0, 0):
            st()
        k8_chain(1)
        for tt in range(8):
            for st in v_chain_steps(tt, 0):
                st()

        # ---- main unit loop ----
        for u in range(NU):
            for s in range(NTT):
                for d in dma_sched.get((u, s), ()):
                    d()
                # ctx columns first: they unblock this slot's pT ring alloc
                for (cu, c, first, last) in ctx_sched.get((u, s), ()):
                    emit_ctx_col(cu, c, first, last)
                emit_score(u, s)
                emit_exp_mask(u, s)
                for st in fillers.get((u, s), ()):
                    st()
                for cu in evac_sched.get((u, s), ()):
                    emit_ctx_evac(cu)
        # tail: last two ctx(15) columns + its evacuation
        for c in (14, 15):
            emit_ctx_col(15, c, 0, 15)
        emit_ctx_evac(15)


def _build():
    global _nc_cache
    if _nc_cache is not None:
        return _nc_cache
    nc = bacc.Bacc(
        "TRN2",
        target_bir_lowering=False,
        debug=False,
        enable_asserts=False,
        num_devices=NCORES,
    )
    xT = nc.dram_tensor("xT", [C, F], BF16, kind="ExternalInput").ap()
    yT = nc.dram_tensor("yT", [C, T], BF16, kind="ExternalInput").ap()
    maskT = nc.dram_tensor("maskT", [T, F], BF16, kind="ExternalInput").ap()
    wq = nc.dram_tensor("wq", [C, COLS], BF16, kind="ExternalInput").ap()
    wk = nc.dram_tensor("wk", [C, COLS], BF16, kind="ExternalInput").ap()
    wv = nc.dram_tensor("wv", [C, COLS], BF16, kind="ExternalInput").ap()
    bq = nc.dram_tensor("bq", [128, 4], F32, kind="ExternalInput").ap()
    bk = nc.dram_tensor("bk", [128, 4], F32, kind="ExternalInput").ap()
    bv = nc.dram_tensor("bv", [1, COLS], BF16, kind="ExternalInput").ap()
    out = nc.dram_tensor("out", [F, 4 * 130], F32, kind="ExternalOutput").ap()

    with tile.TileContext(nc) as tc:
        _emit(tc, nc, (xT, yT, maskT, wq, wk, wv, bq, bk, bv, out))
    nc.compile()
    _nc_cache = nc
    return nc


def _kperm(hg):
    """Local K column (pair*128 + hh*64 + d) -> global Wk column d*H + h_g."""
    idx = np.empty(COLS, dtype=np.int64)
    for pair in range(4):
        for hh in range(2):
            h_g = hg * HL + pair * 2 + hh
            for d in range(DH):
                idx[pair * 128 + hh * 64 + d] = d * H + h_g
    return idx


def make_in_maps(from_tensor, to_tensor, mask, Wq, bq, Wk, bk, Wv, bv):
    per_b = {}
    for b in range(B):
        per_b[b] = (
            np.ascontiguousarray(from_tensor[b].T).astype(bf16),
            np.ascontiguousarray(to_tensor[b].T).astype(bf16),
            np.ascontiguousarray(mask[b].T).astype(bf16),
        )
    in_maps = []
    for i in range(NCORES):
        b, hg = i // 2, i % 2
        xTb, yTb, mTb = per_b[b]
        sl = slice(hg * COLS, (hg + 1) * COLS)
        kidx = _kperm(hg)
        in_maps.append(
            {
                "xT": xTb,
                "yT": yTb,
                "maskT": mTb,
                "wq": np.ascontiguousarray(Wq[:, sl]).astype(bf16),
                "wk": np.ascontiguousarray(Wk[:, kidx]).astype(bf16),
                "wv": np.ascontiguousarray(Wv[:, sl]).astype(bf16),
                "bq": np.ascontiguousarray(
                    bq[sl].astype(np.float32).reshape(4, 128).T
                ),
                "bk": np.ascontiguousarray(
                    bk[kidx].astype(np.float32).reshape(4, 128).T
                ),
                "bv": bv[sl].astype(bf16).reshape(1, COLS),
            }
        )
    return in_maps


def kernel(from_tensor, to_tensor, mask, Wq, bq, Wk, bk, Wv, bv):
    global LAST_RESULTS
    from_tensor = np.asarray(from_tensor, dtype=np.float32)
    to_tensor = np.asarray(to_tensor, dtype=np.float32)
    mask_np = np.asarray(mask)
    Wq = np.asarray(Wq, dtype=np.float32)
    Wk = np.asarray(Wk, dtype=np.float32)
    Wv = np.asarray(Wv, dtype=np.float32)
    bq = np.asarray(bq, dtype=np.float32)
    bk = np.asarray(bk, dtype=np.float32)
    bv = np.asarray(bv, dtype=np.float32)

    nc = _build()
    in_maps = make_in_maps(
        from_tensor, to_tensor, mask_np, Wq, bq, Wk, bk, Wv, bv
    )
    res = bass_utils.run_bass_kernel_spmd(
        nc, in_maps, core_ids=list(range(NCORES)), trace=PROFILE
    )
    LAST_RESULTS = res
    full = np.empty((B, F, H * DH), np.float32)
    for i in range(NCORES):
        b, hg = i // 2, i % 2
        o = res.results[i]["out"].reshape(F, 4, 2, 65)
        ctxv = o[..., :64] / o[..., 64:65]
        full[b, :, hg * COLS:(hg + 1) * COLS] = ctxv.reshape(F, COLS)
    return full


# revision 30
# speedup vs baseline: 1.0858x; 1.0858x over previous
"""Trainium2 Bass kernel for nn_Attention_12137577578573.

Full multi-head attention (QKV projection + masked softmax + context) for
B=4, F=T=2048, CF=CT=1024, H=16, DH=64, sharded over 8 NeuronCores as
(batch b, head-group hg): core i = (b = i // 2, hg = i % 2), each core
computing 1 batch x 8 heads.

v3 schedule (pair-major units for uniform PE load):
  - units iterate u -> (pair = u//4, fc = u%4) instead of (fc, pair).
    With fc-major order ALL K and V projection chains were front-loaded
    into units 0-3 (~23us/unit PE while ACT idled ~30us); pair-major makes
    the projection-chain deadlines uniform: k(pair) chains spread over the
    previous pair-block, v half-chains over units 0-8, one q chain per
    unit.  Per-unit PE load ~15-17us everywhere.
  - cost: all 4 xT quarters stay resident and each unit needs its own fc's
    mask half-tiles.  Masks stay bf16 (an fp8 mask operand drops the DVE
    multiply to 1 elem/cycle: 624 -> 1372 ns/tile, v3a measured) but are
    re-streamed per unit through a 4-buffer ring (2 MB/unit, prefetched one
    unit ahead, split across both HWDGE rings); pT ring shrinks 32->24,
    kT/qT become rings (3 and 8 bufs).
  - softmax exp: ACT for most tiles; OFF_SLOTS tiles use the DVE
    Schraudolph exponent bit-trick written straight into the pT tile
    (bitcast int16); GPS_SLOTS tiles run their mask-multiply on the
    otherwise idle GPSIMD engine instead of DVE.

Layout strategy (unchanged from v1/v2):
  - host pre-transposes from/to tensors -> xT/yT [C, F or T] so QKV
    projections contract C on partitions; Q^T/K^T in transposed layout so
    scores contract DH on partitions; 2 heads packed per 128-partition tile
    (concurrent row-group matmuls).  Scores come out S^T [T, F].  Mask folded
    multiplicatively after exp.  Context C = P^T.T @ [V | 1] gives the
    softmax denominator for free; normalized via per-partition reciprocal.
  - reference reshapes K as (T, DH, H); handled by host-side column
    permutation of Wk/bk.
"""

import sys

if "/opt/trn_rl_repo" not in sys.path:
    sys.path.insert(0, "/opt/trn_rl_repo")

import contextlib

import numpy as np
import ml_dtypes

import concourse.bass as bass
import concourse.bacc as bacc
import concourse.mybir as mybir
import concourse.tile as tile
from concourse import bass_utils

BF16 = mybir.dt.bfloat16
F32 = mybir.dt.float32
F8 = mybir.dt.float8e4
I16 = mybir.dt.int16
bf16 = ml_dtypes.bfloat16
f8 = ml_dtypes.float8_e4m3fn

B, F, T, C, H, DH = 4, 2048, 2048, 1024, 16, 64
HL = 8          # heads per core
COLS = HL * DH  # 512 projected columns per core
ALPHA = 0.125   # 1/sqrt(64)
NCORES = 8
KT = C // 128   # 8 contraction tiles for projections
NFT = F // 128  # 16 F tiles
NTT = T // 128  # 16 T tiles
NU = 16         # units: (pair, fc)

LOG2E = 1.4426950408889634
SCHR_A = ALPHA * LOG2E * 128.0       # score -> bf16-exponent-lattice scale
SCHR_B = 127.0 * 128.0 - 5.0         # exponent bias, mean-error centered

# Schraudolph(DVE)-offloaded exp tile slots per unit (mid-unit so the psum
# drain doesn't stall the next unit's scores).
OFF_SLOTS = [(5, 11)] * NU
# slots whose mask-multiply runs on GPSIMD instead of DVE (measured 2.2us
# per tile on GPSIMD vs 0.63 on DVE -- only worth it if DVE is the wall)
GPS_SLOTS = [()] * NU

PROFILE = False
LAST_RESULTS = None

_nc_cache = None


def _emit(tc, nc, aps):
    xT, yT, maskT, wq, wk, wv, bq, bk, bv, out = aps
    Exp = mybir.ActivationFunctionType.Exp
    Mult = mybir.AluOpType.mult
    Add = mybir.AluOpType.add

    with contextlib.ExitStack() as ctx:
        pool = ctx.enter_context(tc.tile_pool(name="static", bufs=1))
        qTp = ctx.enter_context(tc.tile_pool(name="qTp", bufs=8))
        kTp = ctx.enter_context(tc.tile_pool(name="kTp", bufs=3))
        maskp = ctx.enter_context(tc.tile_pool(name="maskp", bufs=4))
        pTp = ctx.enter_context(tc.tile_pool(name="pTp", bufs=23))
        outp = ctx.enter_context(tc.tile_pool(name="outp", bufs=1))
        psum_s = ctx.enter_context(tc.tile_pool(name="psum_s", bufs=3, space="PSUM"))
        psum_ctx = ctx.enter_context(tc.tile_pool(name="psum_ctx", bufs=2, space="PSUM"))

        # ---- static tiles ----
        yT_sb = pool.tile([128, KT, T], BF16, name="yT_sb", tag="yT_sb")
        wq_sb = pool.tile([128, KT, COLS], BF16, name="wq_sb", tag="wq_sb")
        wk_sb = pool.tile([128, KT, COLS], BF16, name="wk_sb", tag="wk_sb")
        wv_sb = pool.tile([128, KT, COLS], BF16, name="wv_sb", tag="wv_sb")
        xT_sb = pool.tile([128, KT, F], BF16, name="xT_sb", tag="xT_sb")
        v = [pool.tile([128, HL * 65], BF16, name=f"v{tt}", tag=f"v{tt}") for tt in range(NTT)]
        bq_sb = pool.tile([128, 4], F32, name="bq_sb", tag="bq_sb")
        bk_sb = pool.tile([128, 4], F32, name="bk_sb", tag="bk_sb")
        bv_sb = pool.tile([1, COLS], BF16, name="bv_sb", tag="bv_sb")
        ones_sb = pool.tile([1, 128], BF16, name="ones_sb", tag="ones_sb")

        # dram tensors are host-pre-shuffled to partition-major layout so each
        # full-tensor DMA is 128 contiguous descriptors (the DMA queues are
        # descriptor-rate-bound: a [128,8,X]-from-[C,X] load costs 1024
        # descriptors ~60ns each regardless of X)

        # ---- upfront work: two HWDGE DMA rings + engine warmups ----
        # gpsimd (SWDGE, idle): tiny bias loads
        nc.gpsimd.dma_start(bk_sb[:], bk[:])
        nc.gpsimd.dma_start(bq_sb[:], bq[:])
        nc.gpsimd.dma_start(bv_sb[:], bv[:])
        nc.vector.memset(ones_sb[:], 1.0)
        warm_sb = pool.tile([1, 8], F32, name="warm_sb", tag="warm_sb")
        nc.vector.memset(warm_sb[:], 0.0)
        nc.scalar.activation(warm_sb[:], warm_sb[:], Exp)
        # ones columns of all V tiles, written once
        for tt in range(NTT):
            vview = v[tt].rearrange("p (h c) -> p h c", c=65)
            nc.vector.memset(vview[:, :, 64:65], 1.0)

        # sync ring: K/V path in first-use order
        nc.sync.dma_start(wk_sb[:], wk[:])
        nc.sync.dma_start(yT_sb[:], yT[:])
        nc.sync.dma_start(wv_sb[:], wv[:])

        # per-unit mask ring: unit u needs fc = u%4's two half-tiles,
        # prefetched one unit ahead, split across the two HWDGE rings
        mask_u = {}

        def load_mask(u, hf, eng):
            fc = u % 4
            mh = maskp.tile([128, 8, 512], BF16, name="mh", tag="mask")
            eng.dma_start(mh[:], maskT[fc, hf])
            mask_u[(u, hf)] = mh

        def dma_mask(u, hf, eng):
            return lambda: load_mask(u, hf, eng)

        # scalar ring (2nd HWDGE): Q path + first masks.  Everything the
        # scalar ring carries in the prologue delays the ACT engine's first
        # exp (the dma_starts sit in the ACT queue until the ring drains), so
        # keep it light; with partition-major host layout each of these is
        # only 128 descriptors.
        nc.scalar.dma_start(xT_sb[:], xT[:])
        nc.scalar.dma_start(wq_sb[:], wq[:])
        load_mask(0, 0, nc.scalar)
        load_mask(0, 1, nc.scalar)
        load_mask(1, 0, nc.scalar)
        load_mask(1, 1, nc.sync)

        def dma_s(dst, src):
            return lambda: nc.scalar.dma_start(dst, src)

        def dma_y(dst, src):
            return lambda: nc.sync.dma_start(dst, src)

        # ---- projection chains as 2-step closures (4 MM quads + evac) ----
        kT_tiles = {}
        qT_tiles = {}

        def k_chain_steps(pair, tcc):
            st = {}
            def half(i):
                if i == 0:
                    if pair not in kT_tiles:
                        kT_tiles[pair] = kTp.tile(
                            [128, T], BF16, name="kT", tag="kT"
                        )
                    st["ps"] = psum_s.tile([128, 1024], F32, name="ps_f", tag="s")[:, 0:512]
                ps = st["ps"]
                for k in range(4 * i, 4 * i + 4):
                    nc.tensor.matmul(
                        ps[:],
                        wk_sb[:, k, pair * 128:(pair + 1) * 128],
                        yT_sb[:, k, tcc * 512:(tcc + 1) * 512],
                        start=(k == 0),
                        stop=(k == KT - 1),
                    )
                if i == 1:
                    nc.vector.tensor_scalar_add(
                        kT_tiles[pair][:, tcc * 512:(tcc + 1) * 512],
                        ps,
                        bk_sb[:, pair:pair + 1],
                    )
            return [lambda i=i: half(i) for i in range(2)]

        def q_chain_steps(fc, cb):
            st = {}
            def half(i):
                if i == 0:
                    qT_tiles[(fc, cb)] = qTp.tile(
                        [128, 512], BF16, name="qTt", tag="qT"
                    )
                    st["ps"] = psum_s.tile([128, 1024], F32, name="ps_f", tag="s")[:, 0:512]
                ps = st["ps"]
                for k in range(4 * i, 4 * i + 4):
                    nc.tensor.matmul(
                        ps[:],
                        wq_sb[:, k, cb * 128:(cb + 1) * 128],
                        xT_sb[:, k, fc * 512:(fc + 1) * 512],
                        start=(k == 0),
                        stop=(k == KT - 1),
                    )
                if i == 1:
                    nc.vector.tensor_scalar_add(
                        qT_tiles[(fc, cb)][:], ps, bq_sb[:, cb:cb + 1]
                    )
            return [lambda i=i: half(i) for i in range(2)]

        def v_chain_steps(tt, vh):
            # half-width V chain: head columns vh*256:(vh+1)*256 = pairs
            # 2vh, 2vh+1
            st = {}
            def half(i):
                if i == 0:
                    st["ps"] = psum_s.tile([128, 1024], F32, name="ps_f", tag="s")[:, 0:256]
                ps = st["ps"]
                for k in range(4 * i, 4 * i + 4):
                    nc.tensor.matmul(
                        ps[:],
                        yT_sb[:, k, tt * 128:(tt + 1) * 128],
                        wv_sb[:, k, vh * 256:(vh + 1) * 256],
                        start=(k == 0),
                        stop=False,
                    )
                if i == 1:
                    nc.tensor.matmul(
                        ps[:], ones_sb[0:1, :], bv_sb[0:1, vh * 256:(vh + 1) * 256],
                        start=False, stop=True,
                    )
                    vview = v[tt].rearrange("p (h c) -> p h c", c=65)
                    nc.vector.tensor_copy(
                        vview[:, vh * 4:(vh + 1) * 4, 0:64],
                        ps.rearrange("p (h c) -> p h c", c=64)[:],
                    )
            return [lambda i=i: half(i) for i in range(2)]

        # ---- scores + exp + mask ----
        score_ps = {}
        pT_t = {}

        def emit_score(u, tt):
            pair, fc = u // 4, u % 4
            ps = psum_s.tile([128, 1024], F32, name="ps_s", tag="s")
            qt = qT_tiles[(fc, pair)]
            kt = kT_tiles[pair]
            for hh in range(2):
                nc.tensor.matmul(
                    ps[:, hh * 512:(hh + 1) * 512],
                    kt[hh * 64:(hh + 1) * 64, tt * 128:(tt + 1) * 128],
                    qt[hh * 64:(hh + 1) * 64, :],
                    start=True,
                    stop=True,
                )
            score_ps[(u, tt)] = ps

        def emit_exp_mask(u, tt):
            fc = u % 4
            ps = score_ps.pop((u, tt))
            pt = pTp.tile([128, 1024], BF16, name="pT", tag="pT")
            pT_t[(u, tt)] = pt
            mh = mask_u[(u, tt // 8)]
            m = mh[:, tt % 8:tt % 8 + 1, :].broadcast_to([128, 2, 512])
            o = pt.rearrange("p (h c) -> p h c", c=512)
            if tt in OFF_SLOTS[u]:
                nc.vector.tensor_scalar(
                    pt[:].bitcast(I16), ps[:], SCHR_A, SCHR_B, op0=Mult, op1=Add
                )
            else:
                nc.scalar.activation(pt[:], ps[:], Exp, scale=ALPHA)
            eng = nc.gpsimd if tt in GPS_SLOTS[u] else nc.vector
            eng.tensor_mul(o[:], o[:], m)

        # ---- context: column-major accumulation ----
        ctx_ps = {}

        def emit_ctx_col(cu, c, first, last):
            pair = cu // 4
            if c == first:
                ctx_ps[cu] = {
                    hh: psum_ctx.tile([128, 512], F32, name="pc", tag="pc")
                    for hh in range(2)
                }
            pt = pT_t[(cu, c)]
            for hh in range(2):
                pc = ctx_ps[cu][hh]
                h = pair * 2 + hh
                for ft in range(4):
                    # start clears the whole PSUM bank's has_written bits, so
                    # it must be set on the bank's FIRST matmul only (ft==0);
                    # ft 1-3 of the first column land on cleared flags and
                    # overwrite, later columns accumulate.
                    nc.tensor.matmul(
                        pc[:, ft * 65:ft * 65 + 65],
                        pt[:, hh * 512 + ft * 128:hh * 512 + (ft + 1) * 128],
                        v[c][:, h * 65:(h + 1) * 65],
                        start=(c == first and ft == 0),
                        stop=(c == last and ft == 3),
                        skip_group_check=True,
                    )

        def emit_ctx_evac(cu):
            # raw numerators + denominators out; normalization on host
            pair, fc = cu // 4, cu % 4
            ot = outp.tile([128, 4, 130], F32, name="outt", tag="out")
            for hh in range(2):
                pc = ctx_ps[cu][hh]
                nc.vector.tensor_copy(
                    ot[:, :, hh * 65:(hh + 1) * 65],
                    pc[:, 0:260].rearrange("p (ft c) -> p ft c", c=65)[:],
                )
            ctx_ps.pop(cu)
            for tt in range(NTT):
                pT_t.pop((cu, tt), None)
            # last unit's DMA is latency-critical: use the HWDGE sync ring
            eng = nc.sync if cu == 15 else nc.gpsimd
            eng.dma_start(out[cu], ot[:])

        # ---- schedules ----
        # ctx columns emitted at (unit, slot): list of (cu, col, first, last)
        ctx_sched = {}

        def add_ctx(w, s, cu, c, first=0, last=15):
            ctx_sched.setdefault((w, s), []).append((cu, c, first, last))

        for w in range(1, 15):          # 1-unit lag steady state
            for s in range(NTT):
                add_ctx(w, s, w - 1, s)
        # tail compression: ctx(14)'s sources are all ready at u15 start, so
        # run it 2 cols/slot in u15's first half; evac it at slot 8, freeing
        # the psum_ctx buffers for ctx(15) to run 2 cols/slot in the second
        # half (only cols 14-15 + evac remain after the loop)
        for s in range(8):
            add_ctx(15, s, 14, 2 * s)
            add_ctx(15, s, 14, 2 * s + 1)
        for c in range(14):
            add_ctx(15, 9 + c // 2, 15, c)

        # evac at (unit, slot) — must come after the cu's last (stop) column
        evac_sched = {}
        for cu in range(14):
            evac_sched.setdefault((cu + 1, 15), []).append(cu)
        evac_sched.setdefault((15, 8), []).append(14)

        # filler chains, slot-addressed.  Hard ordering constraints (PE FIFO:
        # a waiting instruction must never depend on later PE work):
        #   k(pair,tcc) fully emitted before unit 4*pair slot 4*tcc's score
        #   q(fc,cb) fully emitted before unit 4*cb+fc slot 0
        #   v(tt,vh) for vh=0 before ctx(0) col tt at unit 1 slot tt;
        #   vh=1 before ctx(8) col tt at unit 9 slot tt
        fillers = {}

        def place(u, slot_chains):
            # slot_chains: list of (chain_steps, slot0, slot1)
            for steps, s0, s1 in slot_chains:
                fillers.setdefault((u, s0), []).append(steps[0])
                fillers.setdefault((u, s1), []).append(steps[1])

        place(0, [(k_chain_steps(0, 1), 0, 1), (v_chain_steps(8, 0), 2, 3),
                  (k_chain_steps(0, 2), 4, 5), (q_chain_steps(1, 0), 6, 7),
                  (k_chain_steps(0, 3), 8, 9), (v_chain_steps(9, 0), 10, 11),
                  (v_chain_steps(10, 0), 12, 13), (v_chain_steps(11, 0), 14, 15)])
        place(1, [(v_chain_steps(12, 0), 0, 1), (v_chain_steps(13, 0), 2, 3),
                  (v_chain_steps(14, 0), 4, 5), (v_chain_steps(15, 0), 6, 7),
                  (q_chain_steps(2, 0), 8, 9)])
        place(2, [(k_chain_steps(1, 0), 0, 1), (k_chain_steps(1, 1), 4, 5),
                  (q_chain_steps(3, 0), 6, 7)])
        place(3, [(k_chain_steps(1, 2), 0, 1), (k_chain_steps(1, 3), 4, 5),
                  (q_chain_steps(0, 1), 6, 7)])
        place(4, [(k_chain_steps(2, 0), 0, 1), (v_chain_steps(0, 1), 2, 3),
                  (q_chain_steps(1, 1), 6, 7), (v_chain_steps(1, 1), 10, 11),
                  (v_chain_steps(2, 1), 12, 13)])
        place(5, [(k_chain_steps(2, 1), 0, 1), (v_chain_steps(3, 1), 2, 3),
                  (q_chain_steps(2, 1), 6, 7), (v_chain_steps(4, 1), 10, 11),
                  (v_chain_steps(5, 1), 12, 13)])
        place(6, [(k_chain_steps(2, 2), 0, 1), (v_chain_steps(6, 1), 2, 3),
                  (q_chain_steps(3, 1), 6, 7), (v_chain_steps(7, 1), 10, 11),
                  (v_chain_steps(8, 1), 12, 13)])
        place(7, [(k_chain_steps(2, 3), 0, 1), (v_chain_steps(9, 1), 2, 3),
                  (q_chain_steps(0, 2), 6, 7), (v_chain_steps(10, 1), 10, 11),
                  (v_chain_steps(11, 1), 12, 13)])
        place(8, [(k_chain_steps(3, 0), 0, 1), (v_chain_steps(12, 1), 2, 3),
                  (v_chain_steps(13, 1), 4, 5), (q_chain_steps(1, 2), 6, 7),
                  (v_chain_steps(14, 1), 10, 11)])
        place(9, [(k_chain_steps(3, 1), 0, 1), (v_chain_steps(15, 1), 2, 3),
                  (q_chain_steps(2, 2), 6, 7)])
        place(10, [(k_chain_steps(3, 2), 0, 1), (q_chain_steps(3, 2), 6, 7)])
        place(11, [(k_chain_steps(3, 3), 0, 1), (q_chain_steps(0, 3), 6, 7)])
        place(12, [(q_chain_steps(1, 3), 6, 7)])
        place(13, [(q_chain_steps(2, 3), 6, 7)])
        place(14, [(q_chain_steps(3, 3), 6, 7)])

        # mid-stream DMA triggers at (unit, slot): prefetch unit u+1's masks
        # during unit u (one tile per ring); all other inputs load up front
        dma_sched = {}
        for u in range(1, 15):
            dma_sched.setdefault((u, 2), []).append(dma_mask(u + 1, 0, nc.scalar))
            dma_sched.setdefault((u, 6), []).append(dma_mask(u + 1, 1, nc.sync))

        # ---- PE warmup: keep the clock warm during the initial DMA wait ----
        warm_ps = psum_s.tile([128, 1024], F32, name="ps_w", tag="s")
        for _ in range(32):
            nc.tensor.matmul(
                warm_ps[:, 0:128], ones_sb[0:1, :], ones_sb[0:1, :],
                start=True, stop=True,
            )

        # ---- prologue compute: k(0) first half in T-eighths (tt 0-3) so the
        # first score tile only needs the first yT/wk chunks, q(0,0), and the
        # first half of vh0 ----
        def k8_chain(tc8):
            ps = psum_s.tile([128, 1024], F32, name="ps_f", tag="s")[:, 0:256]
            if 0 not in kT_tiles:
                kT_tiles[0] = kTp.tile([128, T], BF16, name="kT", tag="kT")
            for k in range(KT):
                nc.tensor.matmul(
                    ps[:],
                    wk_sb[:, k, 0:128],
                    yT_sb[:, k, tc8 * 256:(tc8 + 1) * 256],
                    start=(k == 0),
                    stop=(k == KT - 1),
                )
            nc.vector.tensor_scalar_add(
                kT_tiles[0][:, tc8 * 256:(tc8 + 1) * 256], ps, bk_sb[:, 0:1]
            )

        k8_chain(0)
        for st in q_chain_steps(0, 0):
            st()
        k8_chain(1)
        for tt in range(8):
            for st in v_chain_steps(tt, 0):
                st()

        # ---- main unit loop ----
        for u in range(NU):
            for s in range(NTT):
                for d in dma_sched.get((u, s), ()):
                    d()
                # ctx columns first: they unblock this slot's pT ring alloc
                for (cu, c, first, last) in ctx_sched.get((u, s), ()):
                    emit_ctx_col(cu, c, first, last)
                emit_score(u, s)
                emit_exp_mask(u, s)
                for st in fillers.get((u, s), ()):
                    st()
                for cu in evac_sched.get((u, s), ()):
                    emit_ctx_evac(cu)
        # tail: last two ctx(15) columns + its evacuation
        for c in (14, 15):
            emit_ctx_col(15, c, 0, 15)
        emit_ctx_evac(15)


def _build():
    global _nc_cache
    if _nc_cache is not None:
        return _nc_cache
    nc = bacc.Bacc(
        "TRN2",
        target_bir_lowering=False,
        debug=False,
        enable_asserts=False,
        num_devices=NCORES,
    )
    # all inputs host-pre-shuffled to partition-major [128, k, cols] layout
    # (contiguous per partition) so DMAs cost 128 descriptors, not 1024
    xT = nc.dram_tensor("xT", [128, KT, F], BF16, kind="ExternalInput").ap()
    yT = nc.dram_tensor("yT", [128, KT, T], BF16, kind="ExternalInput").ap()
    maskT = nc.dram_tensor(
        "maskT", [4, 2, 128, 8, 512], BF16, kind="ExternalInput"
    ).ap()
    wq = nc.dram_tensor("wq", [128, KT, COLS], BF16, kind="ExternalInput").ap()
    wk = nc.dram_tensor("wk", [128, KT, COLS], BF16, kind="ExternalInput").ap()
    wv = nc.dram_tensor("wv", [128, KT, COLS], BF16, kind="ExternalInput").ap()
    bq = nc.dram_tensor("bq", [128, 4], F32, kind="ExternalInput").ap()
    bk = nc.dram_tensor("bk", [128, 4], F32, kind="ExternalInput").ap()
    bv = nc.dram_tensor("bv", [1, COLS], BF16, kind="ExternalInput").ap()
    out = nc.dram_tensor("out", [16, 128, 4, 130], F32, kind="ExternalOutput").ap()

    with tile.TileContext(nc) as tc:
        _emit(tc, nc, (xT, yT, maskT, wq, wk, wv, bq, bk, bv, out))
    nc.compile()
    _nc_cache = nc
    return nc


def _kperm(hg):
    """Local K column (pair*128 + hh*64 + d) -> global Wk column d*H + h_g."""
    idx = np.empty(COLS, dtype=np.int64)
    for pair in range(4):
        for hh in range(2):
            h_g = hg * HL + pair * 2 + hh
            for d in range(DH):
                idx[pair * 128 + hh * 64 + d] = d * H + h_g
    return idx


def _pmajor(a):
    """[C, X] -> [128, KT, X] partition-major, contiguous per partition."""
    return np.ascontiguousarray(
        a.reshape(KT, 128, a.shape[1]).transpose(1, 0, 2)
    )


def make_in_maps(from_tensor, to_tensor, mask, Wq, bq, Wk, bk, Wv, bv):
    per_b = {}
    for b in range(B):
        # mask[b] is [F, T]; kernel wants tiles [fc, hf, p, j, f] where
        # element (p, j, f) = mask[fc*512+f, (hf*8+j)*128+p]
        m = mask[b].astype(bf16).T.reshape(2, 8, 128, 4, 512)
        m5 = np.ascontiguousarray(m.transpose(3, 0, 2, 1, 4))
        per_b[b] = (
            _pmajor(np.ascontiguousarray(from_tensor[b].T).astype(bf16)),
            _pmajor(np.ascontiguousarray(to_tensor[b].T).astype(bf16)),
            m5,
        )
    in_maps = []
    for i in range(NCORES):
        b, hg = i // 2, i % 2
        xTb, yTb, mTb = per_b[b]
        sl = slice(hg * COLS, (hg + 1) * COLS)
        kidx = _kperm(hg)
        in_maps.append(
            {
                "xT": xTb,
                "yT": yTb,
                "maskT": mTb,
                "wq": _pmajor(np.ascontiguousarray(Wq[:, sl]).astype(bf16)),
                "wk": _pmajor(np.ascontiguousarray(Wk[:, kidx]).astype(bf16)),
                "wv": _pmajor(np.ascontiguousarray(Wv[:, sl]).astype(bf16)),
                "bq": np.ascontiguousarray(
                    bq[sl].astype(np.float32).reshape(4, 128).T
                ),
                "bk": np.ascontiguousarray(
                    bk[kidx].astype(np.float32).reshape(4, 128).T
                ),
                "bv": bv[sl].astype(bf16).reshape(1, COLS),
            }
        )
    return in_maps


def kernel(from_tensor, to_tensor, mask, Wq, bq, Wk, bk, Wv, bv):
    global LAST_RESULTS
    from_tensor = np.asarray(from_tensor, dtype=np.float32)
    to_tensor = np.asarray(to_tensor, dtype=np.float32)
    mask_np = np.asarray(mask)
    Wq = np.asarray(Wq, dtype=np.float32)
    Wk = np.asarray(Wk, dtype=np.float32)
    Wv = np.asarray(Wv, dtype=np.float32)
    bq = np.asarray(bq, dtype=np.float32)
    bk = np.asarray(bk, dtype=np.float32)
    bv = np.asarray(bv, dtype=np.float32)

    nc = _build()
    in_maps = make_in_maps(
        from_tensor, to_tensor, mask_np, Wq, bq, Wk, bk, Wv, bv
    )
    res = bass_utils.run_bass_kernel_spmd(
        nc, in_maps, core_ids=list(range(NCORES)), trace=PROFILE
    )
    LAST_RESULTS = res
    full = np.empty((B, F, H * DH), np.float32)
    for i in range(NCORES):
        b, hg = i // 2, i % 2
        o = res.results[i]["out"].reshape(16, 128, 4, 2, 65)
        ctxv = o[..., :64] / o[..., 64:65]  # [cu, p, g, hh, 64]
        for cu in range(16):
            pair, fc = cu // 4, cu % 4
            blk = ctxv[cu].transpose(1, 0, 2, 3).reshape(4, 128, 128)
            for g in range(4):
                full[
                    b,
                    (fc * 4 + g) * 128:(fc * 4 + g + 1) * 128,
                    hg * COLS + pair * 128:hg * COLS + (pair + 1) * 128,
                ] = blk[g]
    return full


# revision 38
# speedup vs baseline: 1.1214x; 1.0328x over previous
"""Trainium2 Bass kernel for nn_Attention_12137577578573.

Full multi-head attention (QKV projection + masked softmax + context) for
B=4, F=T=2048, CF=CT=1024, H=16, DH=64, sharded over 8 NeuronCores as
(batch b, head-group hg): core i = (b = i // 2, hg = i % 2), each core
computing 1 batch x 8 heads.

v3 schedule (pair-major units for uniform PE load):
  - units iterate u -> (pair = u//4, fc = u%4) instead of (fc, pair).
    With fc-major order ALL K and V projection chains were front-loaded
    into units 0-3 (~23us/unit PE while ACT idled ~30us); pair-major makes
    the projection-chain deadlines uniform: k(pair) chains spread over the
    previous pair-block, v half-chains over units 0-8, one q chain per
    unit.  Per-unit PE load ~15-17us everywhere.
  - cost: all 4 xT quarters stay resident and each unit needs its own fc's
    mask half-tiles.  Masks stay bf16 (an fp8 mask operand drops the DVE
    multiply to 1 elem/cycle: 624 -> 1372 ns/tile, v3a measured) but are
    re-streamed per unit through a 4-buffer ring (2 MB/unit, prefetched one
    unit ahead, split across both HWDGE rings); pT ring shrinks 32->24,
    kT/qT become rings (3 and 8 bufs).
  - softmax exp: ACT for most tiles; OFF_SLOTS tiles use the DVE
    Schraudolph exponent bit-trick written straight into the pT tile
    (bitcast int16); GPS_SLOTS tiles run their mask-multiply on the
    otherwise idle GPSIMD engine instead of DVE.

Layout strategy (unchanged from v1/v2):
  - host pre-transposes from/to tensors -> xT/yT [C, F or T] so QKV
    projections contract C on partitions; Q^T/K^T in transposed layout so
    scores contract DH on partitions; 2 heads packed per 128-partition tile
    (concurrent row-group matmuls).  Scores come out S^T [T, F].  Mask folded
    multiplicatively after exp.  Context C = P^T.T @ [V | 1] gives the
    softmax denominator for free; normalized via per-partition reciprocal.
  - reference reshapes K as (T, DH, H); handled by host-side column
    permutation of Wk/bk.
"""

import sys

if "/opt/trn_rl_repo" not in sys.path:
    sys.path.insert(0, "/opt/trn_rl_repo")

import contextlib

import numpy as np
import ml_dtypes

import concourse.bass as bass
import concourse.bacc as bacc
import concourse.mybir as mybir
import concourse.tile as tile
from concourse import bass_utils

BF16 = mybir.dt.bfloat16
F32 = mybir.dt.float32
F8 = mybir.dt.float8e4
I16 = mybir.dt.int16
bf16 = ml_dtypes.bfloat16
f8 = ml_dtypes.float8_e4m3fn

B, F, T, C, H, DH = 4, 2048, 2048, 1024, 16, 64
HL = 8          # heads per core
COLS = HL * DH  # 512 projected columns per core
ALPHA = 0.125   # 1/sqrt(64)
NCORES = 8
KT = C // 128   # 8 contraction tiles for projections
NFT = F // 128  # 16 F tiles
NTT = T // 128  # 16 T tiles
NU = 16         # units: (pair, fc)

LOG2E = 1.4426950408889634
SCHR_A = ALPHA * LOG2E * 128.0       # score -> bf16-exponent-lattice scale
SCHR_B = 127.0 * 128.0 - 5.0         # exponent bias, mean-error centered

# Schraudolph(DVE)-offloaded exp tile slots per unit (mid-unit so the psum
# drain doesn't stall the next unit's scores).
OFF_SLOTS = [(5, 11)] * NU
# slots whose mask-multiply runs on GPSIMD instead of DVE (measured 2.2us
# per tile on GPSIMD vs 0.63 on DVE -- only worth it if DVE is the wall)
GPS_SLOTS = [()] * NU

PROFILE = False
LAST_RESULTS = None

_nc_cache = None


def _emit(tc, nc, aps):
    xT, yT, maskT, wq, wk, wv, bq, bk, bv, out = aps
    Exp = mybir.ActivationFunctionType.Exp
    Mult = mybir.AluOpType.mult
    Add = mybir.AluOpType.add

    with contextlib.ExitStack() as ctx:
        pool = ctx.enter_context(tc.tile_pool(name="static", bufs=1))
        qTp = ctx.enter_context(tc.tile_pool(name="qTp", bufs=8))
        kTp = ctx.enter_context(tc.tile_pool(name="kTp", bufs=3))
        maskp = ctx.enter_context(tc.tile_pool(name="maskp", bufs=4))
        pTp = ctx.enter_context(tc.tile_pool(name="pTp", bufs=23))
        outp = ctx.enter_context(tc.tile_pool(name="outp", bufs=1))
        psum_s = ctx.enter_context(tc.tile_pool(name="psum_s", bufs=3, space="PSUM"))
        psum_ctx = ctx.enter_context(tc.tile_pool(name="psum_ctx", bufs=2, space="PSUM"))

        # ---- static tiles (one per host-side block so every DMA is 128
        # contiguous descriptors; slicing a big tile re-fragments them) ----
        yT_sb = [pool.tile([128, KT, 512], BF16, name=f"yT{j}", tag=f"yT{j}")
                 for j in range(4)]
        xT_sb = [pool.tile([128, KT, 512], BF16, name=f"xT{j}", tag=f"xT{j}")
                 for j in range(4)]
        wq_sb = pool.tile([128, KT, COLS], BF16, name="wq_sb", tag="wq_sb")
        wk_sb = pool.tile([128, KT, COLS], BF16, name="wk_sb", tag="wk_sb")
        wv_sb = pool.tile([128, KT, COLS], BF16, name="wv_sb", tag="wv_sb")
        v = [pool.tile([128, HL * 65], BF16, name=f"v{tt}", tag=f"v{tt}") for tt in range(NTT)]
        bq_sb = pool.tile([128, 4], F32, name="bq_sb", tag="bq_sb")
        bk_sb = pool.tile([128, 4], F32, name="bk_sb", tag="bk_sb")
        bv_sb = pool.tile([1, COLS], BF16, name="bv_sb", tag="bv_sb")
        ones_sb = pool.tile([1, 128], BF16, name="ones_sb", tag="ones_sb")

        # dram tensors are host-pre-shuffled to partition-major layout so each
        # full-tensor DMA is 128 contiguous descriptors (the DMA queues are
        # descriptor-rate-bound: a [128,8,X]-from-[C,X] load costs 1024
        # descriptors ~60ns each regardless of X)

        # ---- upfront work: two HWDGE DMA rings + engine warmups ----
        # gpsimd (SWDGE, idle): tiny bias loads
        nc.gpsimd.dma_start(bk_sb[:], bk[:])
        nc.gpsimd.dma_start(bq_sb[:], bq[:])
        nc.gpsimd.dma_start(bv_sb[:], bv[:])
        nc.vector.memset(ones_sb[:], 1.0)
        warm_sb = pool.tile([1, 8], F32, name="warm_sb", tag="warm_sb")
        nc.vector.memset(warm_sb[:], 0.0)
        nc.scalar.activation(warm_sb[:], warm_sb[:], Exp)
        # ones columns of all V tiles, written once
        for tt in range(NTT):
            vview = v[tt].rearrange("p (h c) -> p h c", c=65)
            nc.vector.memset(vview[:, :, 64:65], 1.0)

        # per-unit mask ring: unit u needs fc = u%4's two half-tiles,
        # prefetched one unit ahead, split across the two HWDGE rings
        mask_u = {}

        def load_mask(u, hf, eng):
            fc = u % 4
            mh = maskp.tile([128, 8, 512], BF16, name="mh", tag="mask")
            eng.dma_start(mh[:], maskT[fc, hf])
            mask_u[(u, hf)] = mh

        def dma_mask(u, hf, eng):
            return lambda: load_mask(u, hf, eng)

        # scalar ring (2nd HWDGE): Q path + first masks.  Everything the
        # scalar ring carries in the prologue delays the ACT engine's first
        # exp (the dma_starts sit in the ACT queue until the ring drains), so
        # keep it light and deadline-ordered.
        nc.scalar.dma_start(xT_sb[0][:], xT[0])
        nc.scalar.dma_start(wq_sb[:], wq[:])

        # sync ring: K/V path, in first-use order.  Every HWDGE transfer in
        # this kernel is the same [128,8,512] 1MB shape: mixed shapes fan out
        # to different HW queue subsets and complete out of order, which
        # breaks the counting-semaphore waits (observed as first-exec reads
        # of not-yet-landed tiles).
        nc.sync.dma_start(wk_sb[:], wk[:])
        nc.sync.dma_start(yT_sb[0][:], yT[0])
        nc.sync.dma_start(wv_sb[:], wv[:])
        nc.sync.dma_start(yT_sb[1][:], yT[1])
        nc.sync.dma_start(yT_sb[2][:], yT[2])
        nc.sync.dma_start(yT_sb[3][:], yT[3])

        def dma_s(dst, src):
            return lambda: nc.scalar.dma_start(dst, src)

        def dma_y(dst, src):
            return lambda: nc.sync.dma_start(dst, src)

        # ---- projection chains as 2-step closures (4 MM quads + evac) ----
        kT_tiles = {}
        qT_tiles = {}

        def k_chain_steps(pair, tcc):
            st = {}
            def half(i):
                if i == 0:
                    if pair not in kT_tiles:
                        kT_tiles[pair] = kTp.tile(
                            [128, T], BF16, name="kT", tag="kT"
                        )
                    st["ps"] = psum_s.tile([128, 1024], F32, name="ps_f", tag="s")[:, 0:512]
                ps = st["ps"]
                for k in range(4 * i, 4 * i + 4):
                    nc.tensor.matmul(
                        ps[:],
                        wk_sb[:, k, pair * 128:(pair + 1) * 128],
                        yT_sb[tcc][:, k, :],
                        start=(k == 0),
                        stop=(k == KT - 1),
                    )
                if i == 1:
                    nc.vector.tensor_scalar_add(
                        kT_tiles[pair][:, tcc * 512:(tcc + 1) * 512],
                        ps,
                        bk_sb[:, pair:pair + 1],
                    )
            return [lambda i=i: half(i) for i in range(2)]

        def q_chain_steps(fc, cb):
            st = {}
            def half(i):
                if i == 0:
                    qT_tiles[(fc, cb)] = qTp.tile(
                        [128, 512], BF16, name="qTt", tag="qT"
                    )
                    st["ps"] = psum_s.tile([128, 1024], F32, name="ps_f", tag="s")[:, 0:512]
                ps = st["ps"]
                for k in range(4 * i, 4 * i + 4):
                    nc.tensor.matmul(
                        ps[:],
                        wq_sb[:, k, cb * 128:(cb + 1) * 128],
                        xT_sb[fc][:, k, :],
                        start=(k == 0),
                        stop=(k == KT - 1),
                    )
                if i == 1:
                    nc.vector.tensor_scalar_add(
                        qT_tiles[(fc, cb)][:], ps, bq_sb[:, cb:cb + 1]
                    )
            return [lambda i=i: half(i) for i in range(2)]

        def v_chain_steps(tt, vh):
            # half-width V chain: head columns vh*256:(vh+1)*256 = pairs
            # 2vh, 2vh+1
            st = {}
            def half(i):
                if i == 0:
                    st["ps"] = psum_s.tile([128, 1024], F32, name="ps_f", tag="s")[:, 0:256]
                ps = st["ps"]
                for k in range(4 * i, 4 * i + 4):
                    nc.tensor.matmul(
                        ps[:],
                        yT_sb[tt // 4][:, k, (tt % 4) * 128:(tt % 4 + 1) * 128],
                        wv_sb[:, k, vh * 256:(vh + 1) * 256],
                        start=(k == 0),
                        stop=False,
                    )
                if i == 1:
                    nc.tensor.matmul(
                        ps[:], ones_sb[0:1, :], bv_sb[0:1, vh * 256:(vh + 1) * 256],
                        start=False, stop=True,
                    )
                    vview = v[tt].rearrange("p (h c) -> p h c", c=65)
                    nc.vector.tensor_copy(
                        vview[:, vh * 4:(vh + 1) * 4, 0:64],
                        ps.rearrange("p (h c) -> p h c", c=64)[:],
                    )
            return [lambda i=i: half(i) for i in range(2)]

        # ---- scores + exp + mask ----
        score_ps = {}
        pT_t = {}

        def emit_score(u, tt):
            pair, fc = u // 4, u % 4
            ps = psum_s.tile([128, 1024], F32, name="ps_s", tag="s")
            qt = qT_tiles[(fc, pair)]
            kt = kT_tiles[pair]
            for hh in range(2):
                nc.tensor.matmul(
                    ps[:, hh * 512:(hh + 1) * 512],
                    kt[hh * 64:(hh + 1) * 64, tt * 128:(tt + 1) * 128],
                    qt[hh * 64:(hh + 1) * 64, :],
                    start=True,
                    stop=True,
                )
            score_ps[(u, tt)] = ps

        def emit_exp_mask(u, tt):
            fc = u % 4
            ps = score_ps.pop((u, tt))
            pt = pTp.tile([128, 1024], BF16, name="pT", tag="pT")
            pT_t[(u, tt)] = pt
            mh = mask_u[(u, tt // 8)]
            m = mh[:, tt % 8:tt % 8 + 1, :].broadcast_to([128, 2, 512])
            o = pt.rearrange("p (h c) -> p h c", c=512)
            if tt in OFF_SLOTS[u]:
                nc.vector.tensor_scalar(
                    pt[:].bitcast(I16), ps[:], SCHR_A, SCHR_B, op0=Mult, op1=Add
                )
            else:
                nc.scalar.activation(pt[:], ps[:], Exp, scale=ALPHA)
            eng = nc.gpsimd if tt in GPS_SLOTS[u] else nc.vector
            eng.tensor_mul(o[:], o[:], m)

        # ---- context: column-major accumulation ----
        ctx_ps = {}

        def emit_ctx_col(cu, c, first, last):
            pair = cu // 4
            if c == first:
                ctx_ps[cu] = {
                    hh: psum_ctx.tile([128, 512], F32, name="pc", tag="pc")
                    for hh in range(2)
                }
            pt = pT_t[(cu, c)]
            for hh in range(2):
                pc = ctx_ps[cu][hh]
                h = pair * 2 + hh
                for ft in range(4):
                    # start clears the whole PSUM bank's has_written bits, so
                    # it must be set on the bank's FIRST matmul only (ft==0);
                    # ft 1-3 of the first column land on cleared flags and
                    # overwrite, later columns accumulate.
                    nc.tensor.matmul(
                        pc[:, ft * 65:ft * 65 + 65],
                        pt[:, hh * 512 + ft * 128:hh * 512 + (ft + 1) * 128],
                        v[c][:, h * 65:(h + 1) * 65],
                        start=(c == first and ft == 0),
                        stop=(c == last and ft == 3),
                        skip_group_check=True,
                    )

        def emit_ctx_evac(cu):
            # raw numerators + denominators out; normalization on host
            pair, fc = cu // 4, cu % 4
            ot = outp.tile([128, 4, 130], F32, name="outt", tag="out")
            for hh in range(2):
                pc = ctx_ps[cu][hh]
                nc.vector.tensor_copy(
                    ot[:, :, hh * 65:(hh + 1) * 65],
                    pc[:, 0:260].rearrange("p (ft c) -> p ft c", c=65)[:],
                )
            ctx_ps.pop(cu)
            for tt in range(NTT):
                pT_t.pop((cu, tt), None)
            # last unit's DMA is latency-critical: use the HWDGE sync ring
            eng = nc.sync if cu == 15 else nc.gpsimd
            eng.dma_start(out[cu], ot[:])

        # ---- schedules ----
        # ctx columns emitted at (unit, slot): list of (cu, col, first, last)
        ctx_sched = {}

        def add_ctx(w, s, cu, c, first=0, last=15):
            ctx_sched.setdefault((w, s), []).append((cu, c, first, last))

        for w in range(1, 15):          # 1-unit lag steady state
            for s in range(NTT):
                add_ctx(w, s, w - 1, s)
        # tail compression: ctx(14)'s sources are all ready at u15 start, so
        # run it 2 cols/slot in u15's first half; evac it at slot 8, freeing
        # the psum_ctx buffers for ctx(15) to run 2 cols/slot in the second
        # half (only cols 14-15 + evac remain after the loop)
        for s in range(8):
            add_ctx(15, s, 14, 2 * s)
            add_ctx(15, s, 14, 2 * s + 1)
        for c in range(14):
            add_ctx(15, 9 + c // 2, 15, c)

        # evac at (unit, slot) — must come after the cu's last (stop) column
        evac_sched = {}
        for cu in range(14):
            evac_sched.setdefault((cu + 1, 15), []).append(cu)
        evac_sched.setdefault((15, 8), []).append(14)

        # filler chains, slot-addressed.  Hard ordering constraints (PE FIFO:
        # a waiting instruction must never depend on later PE work):
        #   k(pair,tcc) fully emitted before unit 4*pair slot 4*tcc's score
        #   q(fc,cb) fully emitted before unit 4*cb+fc slot 0
        #   v(tt,vh) for vh=0 before ctx(0) col tt at unit 1 slot tt;
        #   vh=1 before ctx(8) col tt at unit 9 slot tt
        fillers = {}

        def place(u, slot_chains):
            # slot_chains: list of (chain_steps, slot0, slot1)
            for steps, s0, s1 in slot_chains:
                fillers.setdefault((u, s0), []).append(steps[0])
                fillers.setdefault((u, s1), []).append(steps[1])

        place(0, [(k_chain_steps(0, 1), 0, 1), (v_chain_steps(4, 0), 2, 3),
                  (k_chain_steps(0, 2), 4, 5), (k_chain_steps(0, 3), 6, 7),
                  (v_chain_steps(5, 0), 8, 9), (v_chain_steps(6, 0), 10, 11),
                  (q_chain_steps(1, 0), 12, 13), (v_chain_steps(7, 0), 14, 15)])
        place(1, [(v_chain_steps(8, 0), 0, 1), (v_chain_steps(9, 0), 2, 3),
                  (v_chain_steps(10, 0), 4, 5), (v_chain_steps(11, 0), 6, 7),
                  (v_chain_steps(12, 0), 8, 9), (v_chain_steps(13, 0), 10, 11),
                  (v_chain_steps(14, 0), 12, 13), (v_chain_steps(15, 0), 13, 14),
                  (q_chain_steps(2, 0), 14, 15)])
        place(2, [(k_chain_steps(1, 0), 0, 1), (k_chain_steps(1, 1), 4, 5),
                  (q_chain_steps(3, 0), 10, 11)])
        place(3, [(k_chain_steps(1, 2), 0, 1), (k_chain_steps(1, 3), 4, 5),
                  (q_chain_steps(0, 1), 6, 7)])
        place(4, [(k_chain_steps(2, 0), 0, 1), (v_chain_steps(0, 1), 2, 3),
                  (q_chain_steps(1, 1), 6, 7), (v_chain_steps(1, 1), 10, 11),
                  (v_chain_steps(2, 1), 12, 13)])
        place(5, [(k_chain_steps(2, 1), 0, 1), (v_chain_steps(3, 1), 2, 3),
                  (q_chain_steps(2, 1), 6, 7), (v_chain_steps(4, 1), 10, 11),
                  (v_chain_steps(5, 1), 12, 13)])
        place(6, [(k_chain_steps(2, 2), 0, 1), (v_chain_steps(6, 1), 2, 3),
                  (q_chain_steps(3, 1), 6, 7), (v_chain_steps(7, 1), 10, 11),
                  (v_chain_steps(8, 1), 12, 13)])
        place(7, [(k_chain_steps(2, 3), 0, 1), (v_chain_steps(9, 1), 2, 3),
                  (q_chain_steps(0, 2), 6, 7), (v_chain_steps(10, 1), 10, 11),
                  (v_chain_steps(11, 1), 12, 13)])
        place(8, [(k_chain_steps(3, 0), 0, 1), (v_chain_steps(12, 1), 2, 3),
                  (v_chain_steps(13, 1), 4, 5), (q_chain_steps(1, 2), 6, 7),
                  (v_chain_steps(14, 1), 10, 11)])
        place(9, [(k_chain_steps(3, 1), 0, 1), (v_chain_steps(15, 1), 2, 3),
                  (q_chain_steps(2, 2), 6, 7)])
        place(10, [(k_chain_steps(3, 2), 0, 1), (q_chain_steps(3, 2), 6, 7)])
        place(11, [(k_chain_steps(3, 3), 0, 1), (q_chain_steps(0, 3), 6, 7)])
        place(12, [(q_chain_steps(1, 3), 6, 7)])
        place(13, [(q_chain_steps(2, 3), 6, 7)])
        place(14, [(q_chain_steps(3, 3), 6, 7)])

        # mid-stream DMA triggers at (unit, slot), deadline-ordered.  All
        # on the scalar ring, which is empty after its small prologue -- the
        # sync ring still has bulk in flight and its counting-sem waits are
        # only sound for data arriving before its consumer's slot.
        dma_sched = {
            (0, 0): [dma_mask(0, 0, nc.scalar)],
            (0, 1): [dma_mask(0, 1, nc.scalar)],
            (0, 3): [dma_s(xT_sb[1][:], xT[1])],
            (0, 4): [dma_mask(1, 0, nc.scalar)],
            (0, 8): [dma_mask(1, 1, nc.scalar)],
            (1, 0): [dma_s(xT_sb[2][:], xT[2])],
            (2, 0): [dma_s(xT_sb[3][:], xT[3])],
        }
        # prefetch unit u+1's masks during unit u
        for u in range(1, 15):
            dma_sched.setdefault((u, 2), []).append(dma_mask(u + 1, 0, nc.scalar))
            dma_sched.setdefault((u, 6), []).append(dma_mask(u + 1, 1, nc.scalar))

        # ---- PE warmup: keep the clock warm during the initial DMA wait ----
        warm_ps = psum_s.tile([128, 1024], F32, name="ps_w", tag="s")
        for _ in range(110):
            nc.tensor.matmul(
                warm_ps[:, 0:128], ones_sb[0:1, :], ones_sb[0:1, :],
                start=True, stop=True,
            )

        # ---- prologue compute: k(0) first half in T-eighths (tt 0-3) so the
        # first score tile only needs the first yT/wk chunks, q(0,0), and the
        # first half of vh0 ----
        def k8_chain(tc8):
            ps = psum_s.tile([128, 1024], F32, name="ps_f", tag="s")[:, 0:256]
            if 0 not in kT_tiles:
                kT_tiles[0] = kTp.tile([128, T], BF16, name="kT", tag="kT")
            for k in range(KT):
                nc.tensor.matmul(
                    ps[:],
                    wk_sb[:, k, 0:128],
                    yT_sb[0][:, k, tc8 * 256:(tc8 + 1) * 256],
                    start=(k == 0),
                    stop=(k == KT - 1),
                )
            nc.vector.tensor_scalar_add(
                kT_tiles[0][:, tc8 * 256:(tc8 + 1) * 256], ps, bk_sb[:, 0:1]
            )

        k8_chain(0)
        for st in q_chain_steps(0, 0):
            st()
        k8_chain(1)
        for tt in range(4):
            for st in v_chain_steps(tt, 0):
                st()

        # ---- main unit loop ----
        for u in range(NU):
            for s in range(NTT):
                for d in dma_sched.get((u, s), ()):
                    d()
                # ctx columns first: they unblock this slot's pT ring alloc
                for (cu, c, first, last) in ctx_sched.get((u, s), ()):
                    emit_ctx_col(cu, c, first, last)
                emit_score(u, s)
                emit_exp_mask(u, s)
                for st in fillers.get((u, s), ()):
                    st()
                for cu in evac_sched.get((u, s), ()):
                    emit_ctx_evac(cu)
        # tail: last two ctx(15) columns + its evacuation
        for c in (14, 15):
            emit_ctx_col(15, c, 0, 15)
        emit_ctx_evac(15)


def _build():
    global _nc_cache
    if _nc_cache is not None:
        return _nc_cache
    nc = bacc.Bacc(
        "TRN2",
        target_bir_lowering=False,
        debug=False,
        enable_asserts=False,
        num_devices=NCORES,
    )
    # all inputs host-pre-shuffled to partition-major [128, k, cols] layout
    # (contiguous per partition) so DMAs cost 128 descriptors, not 1024
    xT = nc.dram_tensor("xT", [4, 128, KT, 512], BF16, kind="ExternalInput").ap()
    yT = nc.dram_tensor("yT", [4, 128, KT, 512], BF16, kind="ExternalInput").ap()
    maskT = nc.dram_tensor(
        "maskT", [4, 2, 128, 8, 512], BF16, kind="ExternalInput"
    ).ap()
    wq = nc.dram_tensor("wq", [128, KT, COLS], BF16, kind="ExternalInput").ap()
    wk = nc.dram_tensor("wk", [128, KT, COLS], BF16, kind="ExternalInput").ap()
    wv = nc.dram_tensor("wv", [128, KT, COLS], BF16, kind="ExternalInput").ap()
    bq = nc.dram_tensor("bq", [128, 4], F32, kind="ExternalInput").ap()
    bk = nc.dram_tensor("bk", [128, 4], F32, kind="ExternalInput").ap()
    bv = nc.dram_tensor("bv", [1, COLS], BF16, kind="ExternalInput").ap()
    out = nc.dram_tensor("out", [16, 128, 4, 130], F32, kind="ExternalOutput").ap()

    with tile.TileContext(nc) as tc:
        _emit(tc, nc, (xT, yT, maskT, wq, wk, wv, bq, bk, bv, out))
    nc.compile()
    _nc_cache = nc
    return nc


def _kperm(hg):
    """Local K column (pair*128 + hh*64 + d) -> global Wk column d*H + h_g."""
    idx = np.empty(COLS, dtype=np.int64)
    for pair in range(4):
        for hh in range(2):
            h_g = hg * HL + pair * 2 + hh
            for d in range(DH):
                idx[pair * 128 + hh * 64 + d] = d * H + h_g
    return idx


def _pblk(a, nb):
    """[C, X] -> [nb, 128, KT, X/nb]: column blocks, each partition-major
    with contiguous per-partition bytes (128-descriptor DMAs)."""
    w = a.shape[1] // nb
    return np.ascontiguousarray(
        a.reshape(KT, 128, nb, w).transpose(2, 1, 0, 3)
    )


def make_in_maps(from_tensor, to_tensor, mask, Wq, bq, Wk, bk, Wv, bv):
    per_b = {}
    for b in range(B):
        # mask[b] is [F, T]; kernel wants tiles [fc, hf, p, j, f] where
        # element (p, j, f) = mask[fc*512+f, (hf*8+j)*128+p]
        m = mask[b].astype(bf16).T.reshape(2, 8, 128, 4, 512)
        m5 = np.ascontiguousarray(m.transpose(3, 0, 2, 1, 4))
        per_b[b] = (
            _pblk(np.ascontiguousarray(from_tensor[b].T).astype(bf16), 4),
            _pblk(np.ascontiguousarray(to_tensor[b].T).astype(bf16), 4),
            m5,
        )
    in_maps = []
    for i in range(NCORES):
        b, hg = i // 2, i % 2
        xTb, yTb, mTb = per_b[b]
        sl = slice(hg * COLS, (hg + 1) * COLS)
        kidx = _kperm(hg)
        in_maps.append(
            {
                "xT": xTb,
                "yT": yTb,
                "maskT": mTb,
                "wq": _pblk(np.ascontiguousarray(Wq[:, sl]).astype(bf16), 1)[0],
                "wk": _pblk(np.ascontiguousarray(Wk[:, kidx]).astype(bf16), 1)[0],
                "wv": _pblk(np.ascontiguousarray(Wv[:, sl]).astype(bf16), 1)[0],
                "bq": np.ascontiguousarray(
                    bq[sl].astype(np.float32).reshape(4, 128).T
                ),
                "bk": np.ascontiguousarray(
                    bk[kidx].astype(np.float32).reshape(4, 128).T
                ),
                "bv": bv[sl].astype(bf16).reshape(1, COLS),
            }
        )
    return in_maps


def kernel(from_tensor, to_tensor, mask, Wq, bq, Wk, bk, Wv, bv):
    global LAST_RESULTS
    from_tensor = np.asarray(from_tensor, dtype=np.float32)
    to_tensor = np.asarray(to_tensor, dtype=np.float32)
    mask_np = np.asarray(mask)
    Wq = np.asarray(Wq, dtype=np.float32)
    Wk = np.asarray(Wk, dtype=np.float32)
    Wv = np.asarray(Wv, dtype=np.float32)
    bq = np.asarray(bq, dtype=np.float32)
    bk = np.asarray(bk, dtype=np.float32)
    bv = np.asarray(bv, dtype=np.float32)

    nc = _build()
    in_maps = make_in_maps(
        from_tensor, to_tensor, mask_np, Wq, bq, Wk, bk, Wv, bv
    )
    res = bass_utils.run_bass_kernel_spmd(
        nc, in_maps, core_ids=list(range(NCORES)), trace=PROFILE
    )
    LAST_RESULTS = res
    full = np.empty((B, F, H * DH), np.float32)
    for i in range(NCORES):
        b, hg = i // 2, i % 2
        o = res.results[i]["out"].reshape(16, 128, 4, 2, 65)
        ctxv = o[..., :64] / o[..., 64:65]  # [cu, p, g, hh, 64]
        for cu in range(16):
            pair, fc = cu // 4, cu % 4
            blk = ctxv[cu].transpose(1, 0, 2, 3).reshape(4, 128, 128)
            for g in range(4):
                full[
                    b,
                    (fc * 4 + g) * 128:(fc * 4 + g + 1) * 128,
                    hg * COLS + pair * 128:hg * COLS + (pair + 1) * 128,
                ] = blk[g]
    return full


# revision 39
# speedup vs baseline: 1.1232x; 1.0016x over previous
"""Trainium2 Bass kernel for nn_Attention_12137577578573.

Full multi-head attention (QKV projection + masked softmax + context) for
B=4, F=T=2048, CF=CT=1024, H=16, DH=64, sharded over 8 NeuronCores as
(batch b, head-group hg): core i = (b = i // 2, hg = i % 2), each core
computing 1 batch x 8 heads.

v3 schedule (pair-major units for uniform PE load):
  - units iterate u -> (pair = u//4, fc = u%4) instead of (fc, pair).
    With fc-major order ALL K and V projection chains were front-loaded
    into units 0-3 (~23us/unit PE while ACT idled ~30us); pair-major makes
    the projection-chain deadlines uniform: k(pair) chains spread over the
    previous pair-block, v half-chains over units 0-8, one q chain per
    unit.  Per-unit PE load ~15-17us everywhere.
  - cost: all 4 xT quarters stay resident and each unit needs its own fc's
    mask half-tiles.  Masks stay bf16 (an fp8 mask operand drops the DVE
    multiply to 1 elem/cycle: 624 -> 1372 ns/tile, v3a measured) but are
    re-streamed per unit through a 4-buffer ring (2 MB/unit, prefetched one
    unit ahead, split across both HWDGE rings); pT ring shrinks 32->24,
    kT/qT become rings (3 and 8 bufs).
  - softmax exp: ACT for most tiles; OFF_SLOTS tiles use the DVE
    Schraudolph exponent bit-trick written straight into the pT tile
    (bitcast int16); GPS_SLOTS tiles run their mask-multiply on the
    otherwise idle GPSIMD engine instead of DVE.

Layout strategy (unchanged from v1/v2):
  - host pre-transposes from/to tensors -> xT/yT [C, F or T] so QKV
    projections contract C on partitions; Q^T/K^T in transposed layout so
    scores contract DH on partitions; 2 heads packed per 128-partition tile
    (concurrent row-group matmuls).  Scores come out S^T [T, F].  Mask folded
    multiplicatively after exp.  Context C = P^T.T @ [V | 1] gives the
    softmax denominator for free; normalized via per-partition reciprocal.
  - reference reshapes K as (T, DH, H); handled by host-side column
    permutation of Wk/bk.
"""

import sys

if "/opt/trn_rl_repo" not in sys.path:
    sys.path.insert(0, "/opt/trn_rl_repo")

import contextlib

import numpy as np
import ml_dtypes

import concourse.bass as bass
import concourse.bacc as bacc
import concourse.mybir as mybir
import concourse.tile as tile
from concourse import bass_utils

BF16 = mybir.dt.bfloat16
F32 = mybir.dt.float32
F8 = mybir.dt.float8e4
I16 = mybir.dt.int16
bf16 = ml_dtypes.bfloat16
f8 = ml_dtypes.float8_e4m3fn

B, F, T, C, H, DH = 4, 2048, 2048, 1024, 16, 64
HL = 8          # heads per core
COLS = HL * DH  # 512 projected columns per core
ALPHA = 0.125   # 1/sqrt(64)
NCORES = 8
KT = C // 128   # 8 contraction tiles for projections
NFT = F // 128  # 16 F tiles
NTT = T // 128  # 16 T tiles
NU = 16         # units: (pair, fc)

LOG2E = 1.4426950408889634
SCHR_A = ALPHA * LOG2E * 128.0       # score -> bf16-exponent-lattice scale
SCHR_B = 127.0 * 128.0 - 5.0         # exponent bias, mean-error centered

# Schraudolph(DVE)-offloaded exp tile slots per unit (mid-unit so the psum
# drain doesn't stall the next unit's scores).
OFF_SLOTS = [(5, 11)] * NU
# slots whose mask-multiply runs on GPSIMD instead of DVE (measured 2.2us
# per tile on GPSIMD vs 0.63 on DVE -- only worth it if DVE is the wall)
GPS_SLOTS = [()] * NU

PROFILE = False
LAST_RESULTS = None

_nc_cache = None


def _emit(tc, nc, aps):
    xT, yT, maskT, wq, wk, wv, bq, bk, bv, out = aps
    Exp = mybir.ActivationFunctionType.Exp
    Mult = mybir.AluOpType.mult
    Add = mybir.AluOpType.add

    with contextlib.ExitStack() as ctx:
        pool = ctx.enter_context(tc.tile_pool(name="static", bufs=1))
        qTp = ctx.enter_context(tc.tile_pool(name="qTp", bufs=6))
        kTp = ctx.enter_context(tc.tile_pool(name="kTp", bufs=3))
        maskp = ctx.enter_context(tc.tile_pool(name="maskp", bufs=8))
        pTp = ctx.enter_context(tc.tile_pool(name="pTp", bufs=23))
        outp = ctx.enter_context(tc.tile_pool(name="outp", bufs=1))
        psum_s = ctx.enter_context(tc.tile_pool(name="psum_s", bufs=3, space="PSUM"))
        psum_ctx = ctx.enter_context(tc.tile_pool(name="psum_ctx", bufs=2, space="PSUM"))

        # ---- static tiles (one per host-side block so every DMA is 128
        # contiguous descriptors; slicing a big tile re-fragments them) ----
        yT_sb = pool.tile([128, 8, KT, 256], BF16, name="yT_sb", tag="yT_sb")
        xT_sb = pool.tile([128, 8, KT, 256], BF16, name="xT_sb", tag="xT_sb")
        wq_sb = pool.tile([128, 2, KT, 256], BF16, name="wq_sb", tag="wq_sb")
        wk_sb = pool.tile([128, 2, KT, 256], BF16, name="wk_sb", tag="wk_sb")
        wv_sb = pool.tile([128, 2, KT, 256], BF16, name="wv_sb", tag="wv_sb")
        v = [pool.tile([128, HL * 65], BF16, name=f"v{tt}", tag=f"v{tt}") for tt in range(NTT)]
        bq_sb = pool.tile([128, 4], F32, name="bq_sb", tag="bq_sb")
        bk_sb = pool.tile([128, 4], F32, name="bk_sb", tag="bk_sb")
        bv_sb = pool.tile([1, COLS], BF16, name="bv_sb", tag="bv_sb")
        ones_sb = pool.tile([1, 128], BF16, name="ones_sb", tag="ones_sb")

        # dram tensors are host-pre-shuffled to partition-major layout so each
        # full-tensor DMA is 128 contiguous descriptors (the DMA queues are
        # descriptor-rate-bound: a [128,8,X]-from-[C,X] load costs 1024
        # descriptors ~60ns each regardless of X)

        # ---- upfront work: two HWDGE DMA rings + engine warmups ----
        # gpsimd (SWDGE, idle): tiny bias loads
        nc.gpsimd.dma_start(bk_sb[:], bk[:])
        nc.gpsimd.dma_start(bq_sb[:], bq[:])
        nc.gpsimd.dma_start(bv_sb[:], bv[:])
        nc.vector.memset(ones_sb[:], 1.0)
        warm_sb = pool.tile([1, 8], F32, name="warm_sb", tag="warm_sb")
        nc.vector.memset(warm_sb[:], 0.0)
        nc.scalar.activation(warm_sb[:], warm_sb[:], Exp)
        # ones columns of all V tiles, written once
        for tt in range(NTT):
            vview = v[tt].rearrange("p (h c) -> p h c", c=65)
            nc.vector.memset(vview[:, :, 64:65], 1.0)

        # per-unit mask ring: unit u needs fc = u%4's two half-tiles,
        # prefetched one unit ahead, split across the two HWDGE rings
        mask_u = {}

        def load_mask(u, qtr, eng):
            fc = u % 4
            mh = maskp.tile([128, 4, 512], BF16, name="mh", tag="mask")
            eng.dma_start(mh[:], maskT[fc, qtr])
            mask_u[(u, qtr)] = mh

        def dma_mask(u, qtr, eng):
            return lambda: load_mask(u, qtr, eng)

        # scalar ring (2nd HWDGE): Q path + first masks.  Everything the
        # scalar ring carries in the prologue delays the ACT engine's first
        # exp (the dma_starts sit in the ACT queue until the ring drains), so
        # keep it light and deadline-ordered.
        # Every HWDGE transfer in this kernel is the same 0.5 MB shape
        # (128 descriptors x 4KB): mixed shapes fan out to different HW queue
        # subsets and complete out of order, which breaks the counting-
        # semaphore waits (observed as first-exec reads of not-yet-landed
        # tiles).  Small uniform blocks also get the first k8/q data in fast.
        nc.scalar.dma_start(xT_sb[:, 0], xT[0])
        nc.scalar.dma_start(xT_sb[:, 1], xT[1])
        nc.scalar.dma_start(wq_sb[:, 0], wq[0])
        load_mask(0, 0, nc.scalar)
        load_mask(0, 1, nc.scalar)

        # sync ring: K/V path, in first-use order
        nc.sync.dma_start(wk_sb[:, 0], wk[0])
        nc.sync.dma_start(yT_sb[:, 0], yT[0])
        nc.sync.dma_start(yT_sb[:, 1], yT[1])
        nc.sync.dma_start(wv_sb[:, 0], wv[0])
        nc.sync.dma_start(yT_sb[:, 2], yT[2])
        nc.sync.dma_start(yT_sb[:, 3], yT[3])
        nc.sync.dma_start(yT_sb[:, 4], yT[4])
        nc.sync.dma_start(yT_sb[:, 5], yT[5])
        nc.sync.dma_start(yT_sb[:, 6], yT[6])
        nc.sync.dma_start(yT_sb[:, 7], yT[7])
        nc.sync.dma_start(wk_sb[:, 1], wk[1])
        nc.sync.dma_start(wv_sb[:, 1], wv[1])

        def dma_s(dst, src):
            return lambda: nc.scalar.dma_start(dst, src)

        def dma_y(dst, src):
            return lambda: nc.sync.dma_start(dst, src)

        # ---- projection chains as 2-step closures (4 MM quads + evac) ----
        kT_tiles = {}
        qT_tiles = {}

        def k_chain_steps(pair, tcc):
            st = {}
            def half(i):
                if i == 0:
                    if pair not in kT_tiles:
                        kT_tiles[pair] = kTp.tile(
                            [128, T], BF16, name="kT", tag="kT"
                        )
                    st["ps"] = psum_s.tile([128, 1024], F32, name="ps_f", tag="s")[:, 0:512]
                ps = st["ps"]
                for k in range(4 * i, 4 * i + 4):
                    nc.tensor.matmul(
                        ps[:],
                        wk_sb[:, pair // 2, k, (pair % 2) * 128:(pair % 2 + 1) * 128],
                        yT_sb[:, 2 * tcc:2 * tcc + 2, k, :],
                        start=(k == 0),
                        stop=(k == KT - 1),
                    )
                if i == 1:
                    nc.vector.tensor_scalar_add(
                        kT_tiles[pair][:, tcc * 512:(tcc + 1) * 512],
                        ps,
                        bk_sb[:, pair:pair + 1],
                    )
            return [lambda i=i: half(i) for i in range(2)]

        def q_chain_steps(fc, cb):
            st = {}
            def half(i):
                if i == 0:
                    qT_tiles[(fc, cb)] = qTp.tile(
                        [128, 512], BF16, name="qTt", tag="qT"
                    )
                    st["ps"] = psum_s.tile([128, 1024], F32, name="ps_f", tag="s")[:, 0:512]
                ps = st["ps"]
                for k in range(4 * i, 4 * i + 4):
                    nc.tensor.matmul(
                        ps[:],
                        wq_sb[:, cb // 2, k, (cb % 2) * 128:(cb % 2 + 1) * 128],
                        xT_sb[:, 2 * fc:2 * fc + 2, k, :],
                        start=(k == 0),
                        stop=(k == KT - 1),
                    )
                if i == 1:
                    nc.vector.tensor_scalar_add(
                        qT_tiles[(fc, cb)][:], ps, bq_sb[:, cb:cb + 1]
                    )
            return [lambda i=i: half(i) for i in range(2)]

        def v_chain_steps(tt, vh):
            # half-width V chain: head columns vh*256:(vh+1)*256 = pairs
            # 2vh, 2vh+1
            st = {}
            def half(i):
                if i == 0:
                    st["ps"] = psum_s.tile([128, 1024], F32, name="ps_f", tag="s")[:, 0:256]
                ps = st["ps"]
                for k in range(4 * i, 4 * i + 4):
                    nc.tensor.matmul(
                        ps[:],
                        yT_sb[:, tt // 2, k, (tt % 2) * 128:(tt % 2 + 1) * 128],
                        wv_sb[:, vh, k, :],
                        start=(k == 0),
                        stop=False,
                    )
                if i == 1:
                    nc.tensor.matmul(
                        ps[:], ones_sb[0:1, :], bv_sb[0:1, vh * 256:(vh + 1) * 256],
                        start=False, stop=True,
                    )
                    vview = v[tt].rearrange("p (h c) -> p h c", c=65)
                    nc.vector.tensor_copy(
                        vview[:, vh * 4:(vh + 1) * 4, 0:64],
                        ps.rearrange("p (h c) -> p h c", c=64)[:],
                    )
            return [lambda i=i: half(i) for i in range(2)]

        # ---- scores + exp + mask ----
        score_ps = {}
        pT_t = {}

        def emit_score(u, tt):
            pair, fc = u // 4, u % 4
            ps = psum_s.tile([128, 1024], F32, name="ps_s", tag="s")
            qt = qT_tiles[(fc, pair)]
            kt = kT_tiles[pair]
            for hh in range(2):
                nc.tensor.matmul(
                    ps[:, hh * 512:(hh + 1) * 512],
                    kt[hh * 64:(hh + 1) * 64, tt * 128:(tt + 1) * 128],
                    qt[hh * 64:(hh + 1) * 64, :],
                    start=True,
                    stop=True,
                )
            score_ps[(u, tt)] = ps

        def emit_exp_mask(u, tt):
            fc = u % 4
            ps = score_ps.pop((u, tt))
            pt = pTp.tile([128, 1024], BF16, name="pT", tag="pT")
            pT_t[(u, tt)] = pt
            mh = mask_u[(u, tt // 4)]
            m = mh[:, tt % 4:tt % 4 + 1, :].broadcast_to([128, 2, 512])
            o = pt.rearrange("p (h c) -> p h c", c=512)
            if tt in OFF_SLOTS[u]:
                nc.vector.tensor_scalar(
                    pt[:].bitcast(I16), ps[:], SCHR_A, SCHR_B, op0=Mult, op1=Add
                )
            else:
                nc.scalar.activation(pt[:], ps[:], Exp, scale=ALPHA)
            eng = nc.gpsimd if tt in GPS_SLOTS[u] else nc.vector
            eng.tensor_mul(o[:], o[:], m)

        # ---- context: column-major accumulation ----
        ctx_ps = {}

        def emit_ctx_col(cu, c, first, last):
            pair = cu // 4
            if c == first:
                ctx_ps[cu] = {
                    hh: psum_ctx.tile([128, 512], F32, name="pc", tag="pc")
                    for hh in range(2)
                }
            pt = pT_t[(cu, c)]
            for hh in range(2):
                pc = ctx_ps[cu][hh]
                h = pair * 2 + hh
                for ft in range(4):
                    # start clears the whole PSUM bank's has_written bits, so
                    # it must be set on the bank's FIRST matmul only (ft==0);
                    # ft 1-3 of the first column land on cleared flags and
                    # overwrite, later columns accumulate.
                    nc.tensor.matmul(
                        pc[:, ft * 65:ft * 65 + 65],
                        pt[:, hh * 512 + ft * 128:hh * 512 + (ft + 1) * 128],
                        v[c][:, h * 65:(h + 1) * 65],
                        start=(c == first and ft == 0),
                        stop=(c == last and ft == 3),
                        skip_group_check=True,
                    )

        def emit_ctx_evac(cu):
            # raw numerators + denominators out; normalization on host
            pair, fc = cu // 4, cu % 4
            ot = outp.tile([128, 4, 130], F32, name="outt", tag="out")
            for hh in range(2):
                pc = ctx_ps[cu][hh]
                nc.vector.tensor_copy(
                    ot[:, :, hh * 65:(hh + 1) * 65],
                    pc[:, 0:260].rearrange("p (ft c) -> p ft c", c=65)[:],
                )
            ctx_ps.pop(cu)
            for tt in range(NTT):
                pT_t.pop((cu, tt), None)
            # last unit's DMA is latency-critical: use the HWDGE sync ring
            eng = nc.sync if cu == 15 else nc.gpsimd
            eng.dma_start(out[cu], ot[:])

        # ---- schedules ----
        # ctx columns emitted at (unit, slot): list of (cu, col, first, last)
        ctx_sched = {}

        def add_ctx(w, s, cu, c, first=0, last=15):
            ctx_sched.setdefault((w, s), []).append((cu, c, first, last))

        # cu 0 keeps the plain slot->col mapping (unit 1's slots also emit
        # the JIT v chains, which must precede their ctx consumer); cu>=1
        # doubles up cols at slots 12-13 so the evac (and its out-DMA) gets 2
        # slots of headroom before the next unit reuses the PSUM banks.
        for s in range(NTT):
            add_ctx(1, s, 0, s)
        for w in range(2, 15):
            for s in range(12):
                add_ctx(w, s, w - 1, s)
            add_ctx(w, 12, w - 1, 12)
            add_ctx(w, 12, w - 1, 13)
            add_ctx(w, 13, w - 1, 14)
            add_ctx(w, 13, w - 1, 15)
        # tail compression: ctx(14)'s sources are all ready at u15 start, so
        # run it 2 cols/slot in u15's first half; evac it at slot 8, freeing
        # the psum_ctx buffers for ctx(15) to run 2 cols/slot in the second
        # half (only cols 14-15 + evac remain after the loop)
        for s in range(8):
            add_ctx(15, s, 14, 2 * s)
            add_ctx(15, s, 14, 2 * s + 1)
        for c in range(14):
            add_ctx(15, 9 + c // 2, 15, c)

        # evac at (unit, slot) — must come after the cu's last (stop) column
        evac_sched = {}
        evac_sched.setdefault((1, 15), []).append(0)
        for cu in range(1, 14):
            evac_sched.setdefault((cu + 1, 14), []).append(cu)
        evac_sched.setdefault((15, 8), []).append(14)

        # filler chains, slot-addressed.  Hard ordering constraints (PE FIFO:
        # a waiting instruction must never depend on later PE work):
        #   k(pair,tcc) fully emitted before unit 4*pair slot 4*tcc's score
        #   q(fc,cb) fully emitted before unit 4*cb+fc slot 0
        #   v(tt,vh) for vh=0 before ctx(0) col tt at unit 1 slot tt;
        #   vh=1 before ctx(8) col tt at unit 9 slot tt
        fillers = {}

        def place(u, slot_chains):
            # slot_chains: list of (chain_steps, slot0, slot1)
            for steps, s0, s1 in slot_chains:
                fillers.setdefault((u, s0), []).append(steps[0])
                fillers.setdefault((u, s1), []).append(steps[1])

        place(0, [(k_chain_steps(0, 1), 0, 1), (v_chain_steps(4, 0), 2, 3),
                  (k_chain_steps(0, 2), 4, 5), (k_chain_steps(0, 3), 6, 7),
                  (v_chain_steps(5, 0), 8, 9), (v_chain_steps(6, 0), 10, 11),
                  (q_chain_steps(1, 0), 12, 13), (v_chain_steps(7, 0), 14, 15)])
        place(1, [(v_chain_steps(8, 0), 0, 1), (v_chain_steps(9, 0), 2, 3),
                  (v_chain_steps(10, 0), 4, 5), (v_chain_steps(11, 0), 6, 7),
                  (v_chain_steps(12, 0), 8, 9), (v_chain_steps(13, 0), 10, 11),
                  (v_chain_steps(14, 0), 12, 13), (v_chain_steps(15, 0), 13, 14),
                  (q_chain_steps(2, 0), 14, 15)])
        place(2, [(k_chain_steps(1, 0), 0, 1), (k_chain_steps(1, 1), 4, 5),
                  (q_chain_steps(3, 0), 10, 11)])
        place(3, [(k_chain_steps(1, 2), 0, 1), (k_chain_steps(1, 3), 4, 5),
                  (q_chain_steps(0, 1), 6, 7)])
        place(4, [(k_chain_steps(2, 0), 0, 1), (v_chain_steps(0, 1), 2, 3),
                  (q_chain_steps(1, 1), 6, 7), (v_chain_steps(1, 1), 10, 11),
                  (v_chain_steps(2, 1), 12, 13)])
        place(5, [(k_chain_steps(2, 1), 0, 1), (v_chain_steps(3, 1), 2, 3),
                  (q_chain_steps(2, 1), 6, 7), (v_chain_steps(4, 1), 10, 11),
                  (v_chain_steps(5, 1), 12, 13)])
        place(6, [(k_chain_steps(2, 2), 0, 1), (v_chain_steps(6, 1), 2, 3),
                  (q_chain_steps(3, 1), 6, 7), (v_chain_steps(7, 1), 10, 11),
                  (v_chain_steps(8, 1), 12, 13)])
        place(7, [(k_chain_steps(2, 3), 0, 1), (v_chain_steps(9, 1), 2, 3),
                  (q_chain_steps(0, 2), 6, 7), (v_chain_steps(10, 1), 10, 11),
                  (v_chain_steps(11, 1), 12, 13)])
        place(8, [(k_chain_steps(3, 0), 0, 1), (v_chain_steps(12, 1), 2, 3),
                  (v_chain_steps(13, 1), 4, 5), (q_chain_steps(1, 2), 6, 7),
                  (v_chain_steps(14, 1), 10, 11)])
        place(9, [(k_chain_steps(3, 1), 0, 1), (v_chain_steps(15, 1), 2, 3),
                  (q_chain_steps(2, 2), 6, 7)])
        place(10, [(k_chain_steps(3, 2), 0, 1), (q_chain_steps(3, 2), 6, 7)])
        place(11, [(k_chain_steps(3, 3), 0, 1), (q_chain_steps(0, 3), 6, 7)])
        place(12, [(q_chain_steps(1, 3), 6, 7)])
        place(13, [(q_chain_steps(2, 3), 6, 7)])
        place(14, [(q_chain_steps(3, 3), 6, 7)])

        # mid-stream DMA triggers at (unit, slot), deadline-ordered.  All
        # on the scalar ring, which is empty after its small prologue -- the
        # sync ring still has bulk in flight and its counting-sem waits are
        # only sound for data arriving before its consumer's slot.
        dma_sched = {
            (0, 0): [dma_mask(0, 2, nc.scalar)],
            (0, 1): [dma_mask(0, 3, nc.scalar)],
            (0, 2): [dma_s(xT_sb[:, 2], xT[2])],
            (0, 3): [dma_s(xT_sb[:, 3], xT[3])],
            (0, 4): [dma_mask(1, 0, nc.scalar)],
            (0, 5): [dma_mask(1, 1, nc.scalar)],
            (0, 6): [dma_mask(1, 2, nc.scalar)],
            (0, 7): [dma_mask(1, 3, nc.scalar)],
            (0, 8): [dma_s(xT_sb[:, 4], xT[4])],
            (0, 9): [dma_s(xT_sb[:, 5], xT[5])],
            (1, 0): [dma_s(xT_sb[:, 6], xT[6])],
            (1, 1): [dma_s(xT_sb[:, 7], xT[7])],
            (2, 0): [dma_s(wq_sb[:, 1], wq[1])],
        }
        # prefetch unit u+1's masks during unit u
        for u in range(1, 15):
            dma_sched.setdefault((u, 2), []).append(dma_mask(u + 1, 0, nc.scalar))
            dma_sched.setdefault((u, 6), []).append(dma_mask(u + 1, 1, nc.scalar))
            dma_sched.setdefault((u, 10), []).append(dma_mask(u + 1, 2, nc.scalar))
            dma_sched.setdefault((u, 14), []).append(dma_mask(u + 1, 3, nc.scalar))

        # ---- PE warmup: keep the clock warm during the initial DMA wait ----
        warm_ps = psum_s.tile([128, 1024], F32, name="ps_w", tag="s")
        for _ in range(90):
            nc.tensor.matmul(
                warm_ps[:, 0:128], ones_sb[0:1, :], ones_sb[0:1, :],
                start=True, stop=True,
            )

        # ---- prologue compute: k(0) first half in T-eighths (tt 0-3) so the
        # first score tile only needs the first yT/wk chunks, q(0,0), and the
        # first half of vh0 ----
        def k8_chain(tc8):
            ps = psum_s.tile([128, 1024], F32, name="ps_f", tag="s")[:, 0:256]
            if 0 not in kT_tiles:
                kT_tiles[0] = kTp.tile([128, T], BF16, name="kT", tag="kT")
            for k in range(KT):
                nc.tensor.matmul(
                    ps[:],
                    wk_sb[:, 0, k, 0:128],
                    yT_sb[:, tc8, k, :],
                    start=(k == 0),
                    stop=(k == KT - 1),
                )
            nc.vector.tensor_scalar_add(
                kT_tiles[0][:, tc8 * 256:(tc8 + 1) * 256], ps, bk_sb[:, 0:1]
            )

        k8_chain(0)
        for st in q_chain_steps(0, 0):
            st()
        k8_chain(1)
        for tt in range(4):
            for st in v_chain_steps(tt, 0):
                st()

        # ---- main unit loop ----
        for u in range(NU):
            for s in range(NTT):
                for d in dma_sched.get((u, s), ()):
                    d()
                # ctx columns first: they unblock this slot's pT ring alloc
                for (cu, c, first, last) in ctx_sched.get((u, s), ()):
                    emit_ctx_col(cu, c, first, last)
                emit_score(u, s)
                emit_exp_mask(u, s)
                for st in fillers.get((u, s), ()):
                    st()
                for cu in evac_sched.get((u, s), ()):
                    emit_ctx_evac(cu)
        # tail: last two ctx(15) columns + its evacuation
        for c in (14, 15):
            emit_ctx_col(15, c, 0, 15)
        emit_ctx_evac(15)


def _build():
    global _nc_cache
    if _nc_cache is not None:
        return _nc_cache
    nc = bacc.Bacc(
        "TRN2",
        target_bir_lowering=False,
        debug=False,
        enable_asserts=False,
        num_devices=NCORES,
    )
    # all inputs host-pre-shuffled to partition-major [128, k, cols] layout
    # (contiguous per partition) so DMAs cost 128 descriptors, not 1024
    xT = nc.dram_tensor("xT", [8, 128, KT, 256], BF16, kind="ExternalInput").ap()
    yT = nc.dram_tensor("yT", [8, 128, KT, 256], BF16, kind="ExternalInput").ap()
    maskT = nc.dram_tensor(
        "maskT", [4, 4, 128, 4, 512], BF16, kind="ExternalInput"
    ).ap()
    wq = nc.dram_tensor("wq", [2, 128, KT, 256], BF16, kind="ExternalInput").ap()
    wk = nc.dram_tensor("wk", [2, 128, KT, 256], BF16, kind="ExternalInput").ap()
    wv = nc.dram_tensor("wv", [2, 128, KT, 256], BF16, kind="ExternalInput").ap()
    bq = nc.dram_tensor("bq", [128, 4], F32, kind="ExternalInput").ap()
    bk = nc.dram_tensor("bk", [128, 4], F32, kind="ExternalInput").ap()
    bv = nc.dram_tensor("bv", [1, COLS], BF16, kind="ExternalInput").ap()
    out = nc.dram_tensor("out", [16, 128, 4, 130], F32, kind="ExternalOutput").ap()

    with tile.TileContext(nc) as tc:
        _emit(tc, nc, (xT, yT, maskT, wq, wk, wv, bq, bk, bv, out))
    nc.compile()
    _nc_cache = nc
    return nc


def _kperm(hg):
    """Local K column (pair*128 + hh*64 + d) -> global Wk column d*H + h_g."""
    idx = np.empty(COLS, dtype=np.int64)
    for pair in range(4):
        for hh in range(2):
            h_g = hg * HL + pair * 2 + hh
            for d in range(DH):
                idx[pair * 128 + hh * 64 + d] = d * H + h_g
    return idx


def _pblk(a, nb):
    """[C, X] -> [nb, 128, KT, X/nb]: column blocks, each partition-major
    with contiguous per-partition bytes (128-descriptor DMAs)."""
    w = a.shape[1] // nb
    return np.ascontiguousarray(
        a.reshape(KT, 128, nb, w).transpose(2, 1, 0, 3)
    )


def make_in_maps(from_tensor, to_tensor, mask, Wq, bq, Wk, bk, Wv, bv):
    per_b = {}
    for b in range(B):
        # mask[b] is [F, T]; kernel wants tiles [fc, hf, p, j, f] where
        # element (p, j, f) = mask[fc*512+f, (hf*8+j)*128+p]
        m = mask[b].astype(bf16).T.reshape(4, 4, 128, 4, 512)
        m5 = np.ascontiguousarray(m.transpose(3, 0, 2, 1, 4))
        per_b[b] = (
            _pblk(np.ascontiguousarray(from_tensor[b].T).astype(bf16), 8),
            _pblk(np.ascontiguousarray(to_tensor[b].T).astype(bf16), 8),
            m5,
        )
    in_maps = []
    for i in range(NCORES):
        b, hg = i // 2, i % 2
        xTb, yTb, mTb = per_b[b]
        sl = slice(hg * COLS, (hg + 1) * COLS)
        kidx = _kperm(hg)
        in_maps.append(
            {
                "xT": xTb,
                "yT": yTb,
                "maskT": mTb,
                "wq": _pblk(np.ascontiguousarray(Wq[:, sl]).astype(bf16), 2),
                "wk": _pblk(np.ascontiguousarray(Wk[:, kidx]).astype(bf16), 2),
                "wv": _pblk(np.ascontiguousarray(Wv[:, sl]).astype(bf16), 2),
                "bq": np.ascontiguousarray(
                    bq[sl].astype(np.float32).reshape(4, 128).T
                ),
                "bk": np.ascontiguousarray(
                    bk[kidx].astype(np.float32).reshape(4, 128).T
                ),
                "bv": bv[sl].astype(bf16).reshape(1, COLS),
            }
        )
    return in_maps


def kernel(from_tensor, to_tensor, mask, Wq, bq, Wk, bk, Wv, bv):
    global LAST_RESULTS
    from_tensor = np.asarray(from_tensor, dtype=np.float32)
    to_tensor = np.asarray(to_tensor, dtype=np.float32)
    mask_np = np.asarray(mask)
    Wq = np.asarray(Wq, dtype=np.float32)
    Wk = np.asarray(Wk, dtype=np.float32)
    Wv = np.asarray(Wv, dtype=np.float32)
    bq = np.asarray(bq, dtype=np.float32)
    bk = np.asarray(bk, dtype=np.float32)
    bv = np.asarray(bv, dtype=np.float32)

    nc = _build()
    in_maps = make_in_maps(
        from_tensor, to_tensor, mask_np, Wq, bq, Wk, bk, Wv, bv
    )
    res = bass_utils.run_bass_kernel_spmd(
        nc, in_maps, core_ids=list(range(NCORES)), trace=PROFILE
    )
    LAST_RESULTS = res
    full = np.empty((B, F, H * DH), np.float32)
    for i in range(NCORES):
        b, hg = i // 2, i % 2
        o = res.results[i]["out"].reshape(16, 128, 4, 2, 65)
        ctxv = o[..., :64] / o[..., 64:65]  # [cu, p, g, hh, 64]
        for cu in range(16):
            pair, fc = cu // 4, cu % 4
            blk = ctxv[cu].transpose(1, 0, 2, 3).reshape(4, 128, 128)
            for g in range(4):
                full[
                    b,
                    (fc * 4 + g) * 128:(fc * 4 + g + 1) * 128,
                    hg * COLS + pair * 128:hg * COLS + (pair + 1) * 128,
                ] = blk[g]
    return full
